# revision 1
# baseline (speedup 1.0000x reference)
"""DeepseekV3 MLA attention (B=1, S=2048, D=2048, H=16) on 8 trn2 NeuronCores.

Strategy (tensor-parallel over heads, replicated low-rank projections):
  - every core computes the full q_a / kv_a low-rank projections (+rmsnorm)
    from a host-transposed hidden state, entirely in a "transposed" layout
    (feature dim on partitions, sequence on the free dim) so attention
    operands come out pre-transposed for the PE;
  - each core owns 2 heads: it computes q_b / kv_b for them, causal
    flash-style attention (no max subtraction -- logits are O(1) here), and
    its slice of o_proj, producing a partial [S, D] output;
  - host sums the 8 partials.

All matmuls run in bf16 (fp32 PSUM accumulation); rmsnorm stats, rope and
softmax run in fp32.  RoPE deinterleave + rotate-half are folded into the
weight layout on the host (extra "pre-swapped, sign-folded" weight columns)
so the device only does aligned elementwise mul/adds.
"""

import numpy as np
import ml_dtypes

import concourse.bass as bass
import concourse.mybir as mybir
import concourse.tile as tile
from concourse.bass_utils import run_bass_kernel_spmd

BF16 = ml_dtypes.bfloat16
F32 = mybir.dt.float32
BF = mybir.dt.bfloat16

B, S, D = 1, 2048, 2048
H = 16
N_CORES = 8
HPC = H // N_CORES  # heads per core = 2
Q_LORA = 1536
KV_LORA = 512
NOPE = 128
ROPE = 64
VD = 128
QHD = NOPE + ROPE  # 192
THETA = 50000.0
EPS = 1e-6
SCALE = QHD ** (-0.5)

NQ = 512            # q-chunk (matmul free dim)
NCHUNK = S // NQ    # 4
KT = S // 128       # 16 k-tiles
AF = mybir.ActivationFunctionType

LAST_RESULTS = None
_CACHE = {}


# ----------------------------------------------------------------------------
# host-side weight preparation
# ----------------------------------------------------------------------------

def _deint_perm():
    # deinterleave: out[j] = in[2j] (j<32), in[2(j-32)+1] (j>=32)
    p = np.empty(ROPE, dtype=np.int64)
    p[:32] = 2 * np.arange(32)
    p[32:] = 2 * np.arange(32) + 1
    return p


def _rope_tables(position_ids):
    pos = np.asarray(position_ids).reshape(-1).astype(np.float32)  # [S]
    inv_freq = (1.0 / (THETA ** (np.arange(0, ROPE, 2, dtype=np.float32) / ROPE)))
    freqs = np.outer(pos, inv_freq)  # [S, 32]
    cos32 = np.cos(freqs).T.astype(np.float32)  # [32, S]
    sin32 = np.sin(freqs).T.astype(np.float32)
    cos128 = np.tile(cos32, (4, 1))  # [128, S]
    sin128 = np.tile(sin32, (4, 1))
    return cos128, sin128


def _causal_mask_big():
    # M[dk, u] = 1 if u >= dk + 384 ; slice [:, 384-128*i : 896-128*i]
    # gives the diagonal-block mask indicator(dq >= dk + 128*i)
    dk = np.arange(128)[:, None]
    u = np.arange(1024)[None, :]
    return (u >= dk + 384).astype(BF16)


def _prep_inputs(inputs):
    hidden = np.asarray(inputs["hidden_states"], dtype=np.float32)[0]  # [S, D]
    position_ids = np.asarray(inputs["position_ids"])
    q_a_w = np.asarray(inputs["q_a_w"], dtype=np.float32)        # [1536, D]
    q_a_ln_w = np.asarray(inputs["q_a_ln_w"], dtype=np.float32)  # [1536]
    q_b_w = np.asarray(inputs["q_b_w"], dtype=np.float32)        # [H*192, 1536]
    kv_a_w = np.asarray(inputs["kv_a_w"], dtype=np.float32)      # [576, D]
    kv_a_ln_w = np.asarray(inputs["kv_a_ln_w"], dtype=np.float32)  # [512]
    kv_b_w = np.asarray(inputs["kv_b_w"], dtype=np.float32)      # [H*256, 512]
    o_w = np.asarray(inputs["o_w"], dtype=np.float32)            # [D, H*128]

    dp = _deint_perm()
    dps = dp[(np.arange(ROPE) ^ 32)]          # source index for the swapped term
    sgn = np.where(np.arange(ROPE) < 32, -1.0, 1.0).astype(np.float32)[:, None]

    shared = {}
    shared["hT"] = np.ascontiguousarray(hidden.T).astype(BF16)          # [D, S]
    shared["qaT"] = np.ascontiguousarray(q_a_w.T).astype(BF16)          # [D, 1536]

    # kv_a columns: [ckv 512 | kpe 64 (deint) | kpe2 64 (swap+sign)]
    kva_cols = np.concatenate(
        [kv_a_w[:KV_LORA], kv_a_w[KV_LORA + dp], sgn * kv_a_w[KV_LORA + dps]], axis=0
    )  # [640, D]
    shared["kvaT"] = np.ascontiguousarray(kva_cols.T).astype(BF16)      # [D, 640]

    cos128, sin128 = _rope_tables(position_ids)
    shared["cosb"] = cos128
    shared["sinb"] = sin128
    shared["maskb"] = _causal_mask_big()

    # q_b with ln + scale folded
    qb = q_b_w * q_a_ln_w[None, :] * SCALE  # [H*192, 1536]
    qb = qb.reshape(H, QHD, Q_LORA)
    kvb = (kv_b_w * kv_a_ln_w[None, :]).reshape(H, NOPE + VD, KV_LORA)

    per_core = []
    for c in range(N_CORES):
        h0, h1 = HPC * c, HPC * c + 1
        nope0 = qb[h0, :NOPE]            # [128, 1536]
        nope1 = qb[h1, :NOPE]
        peP = np.concatenate([qb[h0, NOPE + dp], qb[h1, NOPE + dp]], axis=0)  # [128,...]
        pe2P = np.concatenate(
            [sgn * qb[h0, NOPE + dps], sgn * qb[h1, NOPE + dps]], axis=0
        )
        qb_cols = np.concatenate([nope0, nope1, peP, pe2P], axis=0)  # [512, 1536]
        kb_cols = np.concatenate([kvb[h0, :NOPE], kvb[h1, :NOPE]], axis=0)  # [256, 512]
        vb_cols = np.concatenate([kvb[h0, NOPE:], kvb[h1, NOPE:]], axis=0)  # [256, 512]
        o_slice = o_w[:, VD * h0 : VD * (h1 + 1)]  # [D, 256]
        per_core.append(
            {
                "qbT": np.ascontiguousarray(qb_cols.T).astype(BF16),   # [1536, 512]
                "kbT": np.ascontiguousarray(kb_cols.T).astype(BF16),   # [512, 256]
                "vbT": np.ascontiguousarray(vb_cols.T).astype(BF16),   # [512, 256]
                "owT": np.ascontiguousarray(o_slice.T).astype(BF16),   # [256, S... D]
            }
        )
    return shared, per_core


# ----------------------------------------------------------------------------
# numpy simulation of the device program (for host-side validation)
# ----------------------------------------------------------------------------

def _sim_core(shared, pc):
    bf = lambda x: x.astype(BF16).astype(np.float32)
    hT = shared["hT"].astype(np.float32)          # [D, S]
    qaT = shared["qaT"].astype(np.float32)        # [D, 1536]
    kvaT = shared["kvaT"].astype(np.float32)      # [D, 640]
    cos = shared["cosb"]                          # [128, S]
    sin = shared["sinb"]
    qbT = pc["qbT"].astype(np.float32)            # [1536, 512]
    kbT = pc["kbT"].astype(np.float32)            # [512, 256]
    vbT = pc["vbT"].astype(np.float32)            # [512, 256]
    owT = pc["owT"].astype(np.float32)            # [256, D]

    qaTx = qaT.T @ hT                             # [1536, S]
    qaTb = bf(qaTx)                               # bf16 copy used downstream
    ssq = (bf(qaTb * qaTb)).sum(axis=0)           # square in bf16, fp32 sum
    inv = 1.0 / np.sqrt(ssq / Q_LORA + EPS)       # [S]
    qT = qbT.T @ qaTb                             # [512, S]
    qn0 = bf(qT[0:128] * inv)
    qn1 = bf(qT[128:256] * inv)
    pe, pe2 = qT[256:384], qT[384:512]
    qpe = bf((pe * cos + pe2 * sin) * inv)        # [128, S] packed (h0;h1)

    ckvT = kvaT.T @ hT                            # [640, S]
    ckv = ckvT[:KV_LORA]
    ckvb = bf(ckv)
    ssc = (bf(ckvb * ckvb)).sum(axis=0)
    invc = 1.0 / np.sqrt(ssc / KV_LORA + EPS)
    ckvn = bf(ckvb * invc)                        # [512, S]
    kpe, kpe2 = ckvT[512:576], ckvT[576:640]
    kper = bf(kpe * cos[0:64] + kpe2 * sin[0:64])  # [64, S]

    out = np.zeros((S, D), dtype=np.float32)
    for j in range(HPC):
        knT = bf(kbT[:, 128 * j : 128 * (j + 1)].T @ ckvn)   # [128, S]
        v = bf(ckvn.T @ vbT[:, 128 * j : 128 * (j + 1)])     # [S, 128]
        qn = qn0 if j == 0 else qn1
        qp = qpe[64 * j : 64 * (j + 1)]
        scores = knT.T @ qn + kper.T @ qp         # [S(k), S(q)] -> st[k, q]
        st = scores
        kidx = np.arange(S)[:, None]
        qidx = np.arange(S)[None, :]
        p = np.exp(st) * (kidx <= qidx)
        p = bf(p)
        rs = p.sum(axis=0)                        # [q]
        oT = (v.T @ p)                            # [128, q]
        oT = bf(oT * (1.0 / rs))
        out += oT.T @ owT[128 * j : 128 * (j + 1)]
    return out


def sim(inputs):
    shared, per_core = _prep_inputs(inputs)
    out = np.zeros((S, D), dtype=np.float32)
    for c in range(N_CORES):
        out += _sim_core(shared, per_core[c])
    return out.reshape(B, S, D)


# ----------------------------------------------------------------------------
# bass program
# ----------------------------------------------------------------------------

def _split_waits(nc, max_waits=1):
    """This walrus build accepts at most one sem wait per instruction; hoist
    excess waits onto pure-wait EventSemaphore carriers just before it."""
    n_new = 0
    for f in nc.m.functions:
        for blk in f.blocks:
            new_insts = []
            for inst in blk.instructions:
                si = getattr(inst, "sync_info", None)
                waits = list(si.on_wait) if (si is not None and si.on_wait) else []
                if len(waits) > max_waits:
                    extra, keep = waits[:-max_waits], waits[-max_waits:]
                    for w in extra:
                        n_new += 1
                        carrier = mybir.InstEventSemaphore(
                            name=f"ws-{n_new}-{inst.name}",
                            engine=inst.engine,
                            ins=[],
                            outs=[],
                            sync_info=mybir.SyncInfo(on_wait=[w], on_update=[]),
                        )
                        nc.register_instruction(carrier, overwrite=True)
                        new_insts.append(carrier)
                    si.on_wait = keep
                new_insts.append(inst)
            blk.instructions = new_insts
    return n_new


def _build_nc():
    nc = bass.Bass()
    hT = nc.dram_tensor("hT", [D, S], BF, kind="ExternalInput")
    qaT = nc.dram_tensor("qaT", [D, Q_LORA], BF, kind="ExternalInput")
    kvaT = nc.dram_tensor("kvaT", [D, 640], BF, kind="ExternalInput")
    qbT = nc.dram_tensor("qbT", [Q_LORA, 512], BF, kind="ExternalInput")
    kbT = nc.dram_tensor("kbT", [KV_LORA, 256], BF, kind="ExternalInput")
    vbT = nc.dram_tensor("vbT", [KV_LORA, 256], BF, kind="ExternalInput")
    owT = nc.dram_tensor("owT", [2 * VD, D], BF, kind="ExternalInput")
    cosb = nc.dram_tensor("cosb", [128, S], F32, kind="ExternalInput")
    sinb = nc.dram_tensor("sinb", [128, S], F32, kind="ExternalInput")
    maskb = nc.dram_tensor("maskb", [128, 1024], BF, kind="ExternalInput")
    out = nc.dram_tensor("out", [S, D], F32, kind="ExternalOutput")

    QL_T = Q_LORA // 128  # 12
    D_T = D // 128        # 16
    CV_T = KV_LORA // 128  # 4

    with tile.TileContext(nc) as tc:
        with tc.tile_pool(name="persist1", bufs=1) as persist1:
            ones_t = persist1.tile([128, 128], BF, tag="ones")
            eps_t = persist1.tile([128, 1], F32, tag="eps")
            nc.vector.memset(eps_t, EPS)
            nc.vector.memset(ones_t, 1.0)
            qn_T = [persist1.tile([128, S], BF, tag=f"qnT{h}", name=f"qnT{h}") for h in range(HPC)]
            qpeP = persist1.tile([128, S], BF, tag="qpeP")
            qpe1 = persist1.tile([64, S], BF, tag="qpe1")
            ckvn = [persist1.tile([128, S], BF, tag=f"ckvn{i}", name=f"ckvn{i}") for i in range(CV_T)]
            kperLo = persist1.tile([128, S], BF, tag="kperLo")
            kperHi = persist1.tile([128, S], BF, tag="kperHi")
            nc.vector.memset(kperLo[64:128, :], 0.0)
            nc.vector.memset(kperHi[0:64, :], 0.0)

            # ------------- merged stage 1: q & kv paths, one hidden pass -------------
            with tc.tile_pool(name="qaw", bufs=1) as qaw, \
                 tc.tile_pool(name="kvw", bufs=1) as kvw, \
                 tc.tile_pool(name="qbw", bufs=1) as qbw, \
                 tc.tile_pool(name="hx", bufs=2) as hx, \
                 tc.tile_pool(name="qasb", bufs=1) as qasb, \
                 tc.tile_pool(name="cvsb", bufs=1) as cvsb, \
                 tc.tile_pool(name="csp", bufs=2) as csp, \
                 tc.tile_pool(name="sq", bufs=2) as sqp, \
                 tc.tile_pool(name="nrm", bufs=2) as nrm, \
                 tc.tile_pool(name="nrm2", bufs=2) as nrm2, \
                 tc.tile_pool(name="pet", bufs=1) as pet, \
                 tc.tile_pool(name="st_ps", bufs=3, space="PSUM") as st_ps, \
                 tc.tile_pool(name="ssq_ps", bufs=1, space="PSUM") as ssq_ps, \
                 tc.tile_pool(name="ssq2_ps", bufs=1, space="PSUM") as ssq2_ps, \
                 tc.tile_pool(name="qt_ps", bufs=3, space="PSUM") as qt_ps:

                qa_w = qaw.tile([128, D_T, Q_LORA], BF, tag="qaw")
                kva_w = kvw.tile([128, D_T, 640], BF, tag="kvw")
                qb_w = qbw.tile([128, QL_T, 512], BF, tag="qbw")
                for k in range(D_T):
                    nc.sync.dma_start(out=kva_w[:, k, :], in_=kvaT[128 * k : 128 * (k + 1), :])

                for c in range(NCHUNK):
                    cs = slice(NQ * c, NQ * (c + 1))
                    h_t = hx.tile([128, D_T, NQ], BF, tag="h")
                    for k in range(D_T):
                        nc.sync.dma_start(out=h_t[:, k, :], in_=hT[128 * k : 128 * (k + 1), cs])
                    cos_c = csp.tile([128, NQ], F32, tag="cosc")
                    sin_c = csp.tile([128, NQ], F32, tag="sinc")
                    nc.sync.dma_start(out=cos_c, in_=cosb[:, cs])
                    nc.sync.dma_start(out=sin_c, in_=sinb[:, cs])
                    if c == 0:
                        for k in range(D_T):
                            nc.sync.dma_start(out=qa_w[:, k, :], in_=qaT[128 * k : 128 * (k + 1), :])
                        for m in range(QL_T):
                            nc.sync.dma_start(out=qb_w[:, m, :], in_=qbT[128 * m : 128 * (m + 1), :])

                    # ---- kv_a: 4 ckv m-tiles + kpe + kpe2 ----
                    cv_t = cvsb.tile([128, CV_T, NQ], BF, tag="cv")
                    ssc = ssq2_ps.tile([128, NQ], F32, tag="ssc")
                    pe_ps = []
                    for m in range(6):
                        mp = 128 if m < 4 else 64
                        col = slice(128 * m, 128 * m + 128) if m < 4 else \
                            slice(512 + 64 * (m - 4), 512 + 64 * (m - 3))
                        ps = st_ps.tile([mp, NQ], F32, tag="stps")
                        for k in range(D_T):
                            nc.tensor.matmul(
                                ps,
                                kva_w[:, k, col],
                                h_t[:, k, :],
                                start=(k == 0),
                                stop=(k == D_T - 1),
                            )
                        if m < 4:
                            nc.vector.tensor_copy(cv_t[:, m, :], ps)
                            sq = sqp.tile([128, NQ], BF, tag="sq")
                            nc.scalar.activation(out=sq, in_=ps, func=AF.Square)
                            nc.tensor.matmul(
                                ssc, ones_t, sq, start=(m == 0), stop=(m == CV_T - 1)
                            )
                        else:
                            pe_ps.append(ps)

                    # ---- q_a: 12 m-tiles ----
                    qa_t = qasb.tile([128, QL_T, NQ], BF, tag="qa")
                    ssq = ssq_ps.tile([128, NQ], F32, tag="ssq")
                    for m in range(QL_T):
                        ps = st_ps.tile([128, NQ], F32, tag="stps")
                        for k in range(D_T):
                            nc.tensor.matmul(
                                ps,
                                qa_w[:, k, 128 * m : 128 * (m + 1)],
                                h_t[:, k, :],
                                start=(k == 0),
                                stop=(k == D_T - 1),
                            )
                        nc.vector.tensor_copy(qa_t[:, m, :], ps)
                        sq = sqp.tile([128, NQ], BF, tag="sq")
                        nc.scalar.activation(out=sq, in_=ps, func=AF.Square)
                        nc.tensor.matmul(
                            ssq, ones_t, sq, start=(m == 0), stop=(m == QL_T - 1)
                        )

                    # ---- kv norm + kpe rope ----
                    bc2 = nrm2.tile([128, NQ], F32, tag="bc2")
                    nc.scalar.activation(
                        out=bc2, in_=ssc, func=AF.Sqrt, scale=1.0 / KV_LORA, bias=eps_t
                    )
                    nc.vector.reciprocal(bc2, bc2)
                    for i in range(CV_T):
                        nc.vector.tensor_mul(ckvn[i][:, cs], cv_t[:, i, :], bc2)
                    t1 = pet.tile([128, NQ], F32, tag="t1")
                    t2 = pet.tile([128, NQ], F32, tag="t2")
                    nc.vector.tensor_mul(t1[0:64, :], pe_ps[0], cos_c[0:64, :])
                    nc.vector.tensor_mul(t2[0:64, :], pe_ps[1], sin_c[0:64, :])
                    nc.vector.tensor_add(kperLo[0:64, cs], t1[0:64, :], t2[0:64, :])
                    nc.vector.tensor_add(kperHi[64:128, cs], t1[0:64, :], t2[0:64, :])

                    # ---- q_b: 4 col-blocks accumulated over 12 m ----
                    bc = nrm.tile([128, NQ], F32, tag="bc")
                    nc.scalar.activation(
                        out=bc, in_=ssq, func=AF.Sqrt, scale=1.0 / Q_LORA, bias=eps_t
                    )
                    nc.vector.reciprocal(bc, bc)
                    qt_tiles = []
                    for b in range(4):
                        ps = qt_ps.tile([128, NQ], F32, tag="qtps")
                        for m in range(QL_T):
                            nc.tensor.matmul(
                                ps,
                                qb_w[:, m, 128 * b : 128 * (b + 1)],
                                qa_t[:, m, :],
                                start=(m == 0),
                                stop=(m == QL_T - 1),
                            )
                        if b == 0:
                            nc.vector.tensor_mul(qn_T[0][:, cs], ps, bc)
                        elif b == 1:
                            nc.vector.tensor_mul(qn_T[1][:, cs], ps, bc)
                        else:
                            qt_tiles.append(ps)
                    nc.vector.tensor_mul(t1, qt_tiles[0], cos_c)
                    nc.vector.tensor_mul(t2, qt_tiles[1], sin_c)
                    nc.vector.tensor_add(t1, t1, t2)
                    nc.vector.tensor_mul(qpeP[:, cs], t1, bc)
                nc.sync.dma_start(out=qpe1[:, :], in_=qpeP[64:128, :])

            # ---------------- phase B2: kv_b projections ----------------
            with tc.tile_pool(name="persist2", bufs=1) as persist2:
                kn_T = [persist2.tile([128, S], BF, tag=f"knT{h}", name=f"knT{h}") for h in range(HPC)]
                v_sb = [persist2.tile([128, S], BF, tag=f"v{h}", name=f"v{h}") for h in range(HPC)]
                o_T = [persist2.tile([128, S], BF, tag=f"oT{h}", name=f"oT{h}") for h in range(HPC)]
                with tc.tile_pool(name="kbw", bufs=1) as kbw, \
                     tc.tile_pool(name="kn_ps", bufs=2, space="PSUM") as kn_ps, \
                     tc.tile_pool(name="v_ps", bufs=3, space="PSUM") as v_ps:
                    kb_w = kbw.tile([128, CV_T, 256], BF, tag="kbw")
                    vb_w = kbw.tile([128, CV_T, 256], BF, tag="vbw")
                    for ct in range(CV_T):
                        nc.sync.dma_start(out=kb_w[:, ct, :], in_=kbT[128 * ct : 128 * (ct + 1), :])
                        nc.sync.dma_start(out=vb_w[:, ct, :], in_=vbT[128 * ct : 128 * (ct + 1), :])
                    for h in range(HPC):
                        hs = slice(128 * h, 128 * (h + 1))
                        for c in range(NCHUNK):
                            cs = slice(NQ * c, NQ * (c + 1))
                            ps = kn_ps.tile([128, NQ], F32, tag="knps")
                            for ct in range(CV_T):
                                nc.tensor.matmul(
                                    ps,
                                    kb_w[:, ct, hs],
                                    ckvn[ct][:, cs],
                                    start=(ct == 0),
                                    stop=(ct == CV_T - 1),
                                )
                            nc.vector.tensor_copy(kn_T[h][:, cs], ps)
                        for kt in range(KT):
                            ks = slice(128 * kt, 128 * (kt + 1))
                            ps = v_ps.tile([128, VD], F32, tag="vps")
                            for ct in range(CV_T):
                                nc.tensor.matmul(
                                    ps,
                                    ckvn[ct][:, ks],
                                    vb_w[:, ct, hs],
                                    start=(ct == 0),
                                    stop=(ct == CV_T - 1),
                                )
                            nc.vector.tensor_copy(v_sb[h][:, ks], ps)

                # ---------------- phase C: attention ----------------
                mskp_cm = tc.tile_pool(name="mskp", bufs=1)
                oww_cm = tc.tile_pool(name="oww", bufs=1)
                mskp = mskp_cm.__enter__()
                oww = oww_cm.__enter__()
                with tc.tile_pool(name="pp", bufs=6) as pp, \
                     tc.tile_pool(name="ep", bufs=3) as ep, \
                     tc.tile_pool(name="rvp", bufs=2) as rvp, \
                     tc.tile_pool(name="ostg", bufs=4) as ostg, \
                     tc.tile_pool(name="s_ps", bufs=3, space="PSUM") as s_ps, \
                     tc.tile_pool(name="rs_ps", bufs=2, space="PSUM") as rs_ps, \
                     tc.tile_pool(name="o_ps", bufs=2, space="PSUM") as o_ps, \
                     tc.tile_pool(name="out_ps", bufs=1, space="PSUM") as out_ps:
                    mask_s = mskp.tile([128, 1024], BF, tag="mask")
                    nc.sync.dma_start(out=mask_s, in_=maskb[:, :])
                    ow_t = oww.tile([128, HPC, D], BF, tag="oww")
                    for j in range(HPC):
                        nc.sync.dma_start(out=ow_t[:, j, :], in_=owT[128 * j : 128 * (j + 1), :])
                    for c in range(NCHUNK):
                        cs = slice(NQ * c, NQ * (c + 1))
                        nkt = 4 * (c + 1)
                        for h in range(HPC):
                            kper_h = kperLo if h == 0 else kperHi
                            rs = rs_ps.tile([128, NQ], F32, tag="rs")
                            op = o_ps.tile([128, NQ], F32, tag="op")
                            for kt in range(nkt):
                                ks = slice(128 * kt, 128 * (kt + 1))
                                i = kt - 4 * c
                                lo = 128 * i if i > 0 else 0  # valid q-subrange start
                                qs = slice(NQ * c + lo, NQ * (c + 1))
                                vs = slice(lo, NQ)
                                sp = s_ps.tile([128, NQ], F32, tag="sp")
                                nc.tensor.matmul(
                                    sp[:, vs], kn_T[h][:, ks], qn_T[h][:, qs],
                                    start=True, stop=False,
                                )
                                nc.tensor.matmul(
                                    sp[:, vs], kper_h[:, ks], qpeP[:, qs],
                                    start=False, stop=True,
                                )
                                p_t = pp.tile([128, NQ], BF, tag="p")
                                if kt >= 4 * c:
                                    e_t = ep.tile([128, NQ], BF, tag="e")
                                    nc.scalar.activation(out=e_t[:, vs], in_=sp[:, vs], func=AF.Exp)
                                    nc.vector.tensor_mul(
                                        p_t[:, vs], e_t[:, vs],
                                        mask_s[:, 384 : 896 - lo],
                                    )
                                else:
                                    nc.scalar.activation(out=p_t[:, vs], in_=sp[:, vs], func=AF.Exp)
                                nc.tensor.matmul(
                                    rs[:, vs], ones_t, p_t[:, vs],
                                    start=(kt == 0), stop=(kt == nkt - 1),
                                )
                                nc.tensor.matmul(
                                    op[:, vs],
                                    v_sb[h][:, ks],
                                    p_t[:, vs],
                                    start=(kt == 0), stop=(kt == nkt - 1),
                                )
                            rv = rvp.tile([128, NQ], F32, tag="rv")
                            nc.vector.reciprocal(rv, rs)
                            nc.vector.tensor_mul(o_T[h][:, cs], op, rv)
                        # o_proj for this chunk's 4 s-tiles (both heads now done;
                        # last chunk handled in a post-phase with deeper PSUM)
                        for si in range(4 * c, 4 * (c + 1) if c < NCHUNK - 1 else 4 * c):
                            ss = slice(128 * si, 128 * (si + 1))
                            for nch in range(NCHUNK):
                                ns = slice(NQ * nch, NQ * (nch + 1))
                                ps = out_ps.tile([128, NQ], F32, tag="outps")
                                for j in range(HPC):
                                    nc.tensor.matmul(
                                        ps,
                                        o_T[j][:, ss],
                                        ow_t[:, j, ns],
                                        start=(j == 0),
                                        stop=(j == HPC - 1),
                                    )
                                stg = ostg.tile([128, NQ], F32, tag="ostg")
                                nc.scalar.activation(out=stg, in_=ps, func=AF.Copy)
                                nc.sync.dma_start(out=out[ss, ns], in_=stg)
                # ---------------- final chunk o_proj ----------------
                with tc.tile_pool(name="ostg2", bufs=4) as ostg2, \
                     tc.tile_pool(name="out2_ps", bufs=4, space="PSUM") as out2_ps:
                    for si in range(4 * (NCHUNK - 1), 4 * NCHUNK):
                        ss = slice(128 * si, 128 * (si + 1))
                        for nch in range(NCHUNK):
                            ns = slice(NQ * nch, NQ * (nch + 1))
                            ps = out2_ps.tile([128, NQ], F32, tag="out2ps")
                            for j in range(HPC):
                                nc.tensor.matmul(
                                    ps,
                                    o_T[j][:, ss],
                                    ow_t[:, j, ns],
                                    start=(j == 0),
                                    stop=(j == HPC - 1),
                                )
                            stg = ostg2.tile([128, NQ], F32, tag="ostg2")
                            nc.scalar.activation(out=stg, in_=ps, func=AF.Copy)
                            nc.sync.dma_start(out=out[ss, ns], in_=stg)
                oww_cm.__exit__(None, None, None)
                mskp_cm.__exit__(None, None, None)

    _split_waits(nc)
    return nc


# ----------------------------------------------------------------------------
# entry point
# ----------------------------------------------------------------------------

def kernel(**inputs):
    global LAST_RESULTS
    shared, per_core = _prep_inputs(inputs)
    if "nc" not in _CACHE:
        _CACHE["nc"] = _build_nc()
    nc = _CACHE["nc"]
    in_maps = []
    for c in range(N_CORES):
        m = {
            "hT": shared["hT"],
            "qaT": shared["qaT"],
            "kvaT": shared["kvaT"],
            "cosb": shared["cosb"],
            "sinb": shared["sinb"],
            "maskb": shared["maskb"],
            "qbT": per_core[c]["qbT"],
            "kbT": per_core[c]["kbT"],
            "vbT": per_core[c]["vbT"],
            "owT": per_core[c]["owT"],
        }
        in_maps.append(m)
    res = run_bass_kernel_spmd(nc, in_maps, core_ids=list(range(N_CORES)))
    LAST_RESULTS = res
    out = np.zeros((S, D), dtype=np.float32)
    for r in res.results:
        out += r["out"]
    return out.reshape(B, S, D)



# revision 2
# speedup vs baseline: 1.5652x; 1.5652x over previous
"""DeepseekV3 MLA attention (B=1, S=2048, D=2048, H=16) on 8 trn2 NeuronCores.

Strategy (v2 -- collective-sharded stage 1, tensor-parallel attention):
  - stage 1 (q_a / kv_a low-rank projections + rmsnorm + k-rope) is sharded
    over TOKENS: each core processes S/8 = 256 tokens, then two device
    AllGathers (ckv path first, then q path) replicate the small normalized
    activations to every core;
  - each core owns 2 heads: q_b / kv_b projections for them, causal
    flash-style attention (no max subtraction -- logits are O(1) here), and
    its slice of o_proj, producing a partial [S, D] output;
  - host sums the 8 partials.

All matmuls run in bf16 (fp32 PSUM accumulation); rmsnorm stats, rope and
softmax run in fp32.  RoPE deinterleave + rotate-half are folded into the
weight layout on the host (extra "pre-swapped, sign-folded" weight columns)
so the device only does aligned elementwise mul/adds.
"""

import numpy as np
import ml_dtypes

import concourse.bass as bass
import concourse.mybir as mybir
import concourse.tile as tile
from concourse.bass_utils import run_bass_kernel_spmd

BF16 = ml_dtypes.bfloat16
F32 = mybir.dt.float32
BF = mybir.dt.bfloat16

B, S, D = 1, 2048, 2048
H = 16
N_CORES = 8
HPC = H // N_CORES  # heads per core = 2
SLC = S // N_CORES  # stage-1 token slice per core = 256
Q_LORA = 1536
KV_LORA = 512
NOPE = 128
ROPE = 64
VD = 128
QHD = NOPE + ROPE  # 192
THETA = 50000.0
EPS = 1e-6
SCALE = QHD ** (-0.5)

NQ = 512            # q-chunk (matmul free dim)
NCHUNK = S // NQ    # 4
KT = S // 128       # 16 k-tiles
QL_T = Q_LORA // 128  # 12
D_T = D // 128        # 16
CV_T = KV_LORA // 128  # 4
AF = mybir.ActivationFunctionType

LAST_RESULTS = None
_CACHE = {}


# ----------------------------------------------------------------------------
# host-side weight preparation
# ----------------------------------------------------------------------------

def _deint_perm():
    # deinterleave: out[j] = in[2j] (j<32), in[2(j-32)+1] (j>=32)
    p = np.empty(ROPE, dtype=np.int64)
    p[:32] = 2 * np.arange(32)
    p[32:] = 2 * np.arange(32) + 1
    return p


def _rope_tables(position_ids):
    pos = np.asarray(position_ids).reshape(-1).astype(np.float32)  # [S]
    inv_freq = (1.0 / (THETA ** (np.arange(0, ROPE, 2, dtype=np.float32) / ROPE)))
    freqs = np.outer(pos, inv_freq)  # [S, 32]
    cos32 = np.cos(freqs).T.astype(np.float32)  # [32, S]
    sin32 = np.sin(freqs).T.astype(np.float32)
    cos128 = np.tile(cos32, (4, 1))  # [128, S]
    sin128 = np.tile(sin32, (4, 1))
    return cos128, sin128


def _causal_mask_big():
    # M[dk, u] = 1 if u >= dk + 384 ; slice [:, 384-128*i : 896-128*i]
    # gives the diagonal-block mask indicator(dq >= dk + 128*i)
    dk = np.arange(128)[:, None]
    u = np.arange(1024)[None, :]
    return (u >= dk + 384).astype(BF16)


def _prep_inputs(inputs):
    hidden = np.asarray(inputs["hidden_states"], dtype=np.float32)[0]  # [S, D]
    position_ids = np.asarray(inputs["position_ids"])
    q_a_w = np.asarray(inputs["q_a_w"], dtype=np.float32)        # [1536, D]
    q_a_ln_w = np.asarray(inputs["q_a_ln_w"], dtype=np.float32)  # [1536]
    q_b_w = np.asarray(inputs["q_b_w"], dtype=np.float32)        # [H*192, 1536]
    kv_a_w = np.asarray(inputs["kv_a_w"], dtype=np.float32)      # [576, D]
    kv_a_ln_w = np.asarray(inputs["kv_a_ln_w"], dtype=np.float32)  # [512]
    kv_b_w = np.asarray(inputs["kv_b_w"], dtype=np.float32)      # [H*256, 512]
    o_w = np.asarray(inputs["o_w"], dtype=np.float32)            # [D, H*128]

    dp = _deint_perm()
    dps = dp[(np.arange(ROPE) ^ 32)]          # source index for the swapped term
    sgn = np.where(np.arange(ROPE) < 32, -1.0, 1.0).astype(np.float32)[:, None]

    hT = np.ascontiguousarray(hidden.T).astype(BF16)              # [D, S]
    shared = {}
    shared["qaT"] = np.ascontiguousarray(q_a_w.T).astype(BF16)          # [D, 1536]

    # kv_a columns: [ckv 512 | kpe 64 (deint) | kpe2 64 (swap+sign)]
    kva_cols = np.concatenate(
        [kv_a_w[:KV_LORA], kv_a_w[KV_LORA + dp], sgn * kv_a_w[KV_LORA + dps]], axis=0
    )  # [640, D]
    shared["kvaT"] = np.ascontiguousarray(kva_cols.T).astype(BF16)      # [D, 640]

    cos128, sin128 = _rope_tables(position_ids)
    shared["cosb"] = cos128
    shared["sinb"] = sin128
    shared["maskb"] = _causal_mask_big()

    # q_b with ln + scale folded
    qb = q_b_w * q_a_ln_w[None, :] * SCALE  # [H*192, 1536]
    qb = qb.reshape(H, QHD, Q_LORA)
    kvb = (kv_b_w * kv_a_ln_w[None, :]).reshape(H, NOPE + VD, KV_LORA)

    per_core = []
    for c in range(N_CORES):
        h0, h1 = HPC * c, HPC * c + 1
        nope0 = qb[h0, :NOPE]            # [128, 1536]
        nope1 = qb[h1, :NOPE]
        peP = np.concatenate([qb[h0, NOPE + dp], qb[h1, NOPE + dp]], axis=0)  # [128,...]
        pe2P = np.concatenate(
            [sgn * qb[h0, NOPE + dps], sgn * qb[h1, NOPE + dps]], axis=0
        )
        qb_cols = np.concatenate([nope0, nope1, peP, pe2P], axis=0)  # [512, 1536]
        kb_cols = np.concatenate([kvb[h0, :NOPE], kvb[h1, :NOPE]], axis=0)  # [256, 512]
        vb_cols = np.concatenate([kvb[h0, NOPE:], kvb[h1, NOPE:]], axis=0)  # [256, 512]
        o_slice = o_w[:, VD * h0 : VD * (h1 + 1)]  # [D, 256]
        ts = slice(SLC * c, SLC * (c + 1))
        per_core.append(
            {
                "hTs": np.ascontiguousarray(hT[:, ts]),                # [D, 256]
                "cosa": np.ascontiguousarray(cos128[0:64, ts]),        # [64, 256]
                "sina": np.ascontiguousarray(sin128[0:64, ts]),
                "qbT": np.ascontiguousarray(qb_cols.T).astype(BF16),   # [1536, 512]
                "kbT": np.ascontiguousarray(kb_cols.T).astype(BF16),   # [512, 256]
                "vbT": np.ascontiguousarray(vb_cols.T).astype(BF16),   # [512, 256]
                "owT": np.ascontiguousarray(o_slice.T).astype(BF16),   # [256, D]
            }
        )
    return shared, per_core


# ----------------------------------------------------------------------------
# numpy simulation of the device program (for host-side validation)
# ----------------------------------------------------------------------------

def _sim_stage1(shared, per_core):
    bf = lambda x: x.astype(BF16).astype(np.float32)
    qaT = shared["qaT"].astype(np.float32)        # [D, 1536]
    kvaT = shared["kvaT"].astype(np.float32)      # [D, 640]
    cos = shared["cosb"]                          # [128, S]
    sin = shared["sinb"]
    qa_n = np.zeros((Q_LORA, S), dtype=np.float32)
    ckvn = np.zeros((KV_LORA, S), dtype=np.float32)
    kper = np.zeros((ROPE, S), dtype=np.float32)
    for c in range(N_CORES):
        ts = slice(SLC * c, SLC * (c + 1))
        hTs = per_core[c]["hTs"].astype(np.float32)
        qa = qaT.T @ hTs                          # [1536, 256]
        qab = bf(qa)
        ssq = bf(qab * qab).sum(axis=0)
        inv = 1.0 / np.sqrt(ssq / Q_LORA + EPS)
        qa_n[:, ts] = bf(qab * inv)
        ckv = kvaT.T @ hTs                        # [640, 256]
        cb = bf(ckv[:KV_LORA])
        ssc = bf(cb * cb).sum(axis=0)
        invc = 1.0 / np.sqrt(ssc / KV_LORA + EPS)
        ckvn[:, ts] = bf(cb * invc)
        kpe, kpe2 = ckv[512:576], ckv[576:640]
        kper[:, ts] = bf(kpe * cos[0:64, ts] + kpe2 * sin[0:64, ts])
    return qa_n, ckvn, kper


def _sim_core(shared, pc, qa_n, ckvn, kper):
    bf = lambda x: x.astype(BF16).astype(np.float32)
    cos = shared["cosb"]
    sin = shared["sinb"]
    qbT = pc["qbT"].astype(np.float32)            # [1536, 512]
    kbT = pc["kbT"].astype(np.float32)            # [512, 256]
    vbT = pc["vbT"].astype(np.float32)            # [512, 256]
    owT = pc["owT"].astype(np.float32)            # [256, D]

    qT = qbT.T @ qa_n                             # [512, S]
    qn0 = bf(qT[0:128])
    qn1 = bf(qT[128:256])
    pe, pe2 = qT[256:384], qT[384:512]
    qpe = bf(pe * cos + pe2 * sin)                # [128, S] packed (h0;h1)

    out = np.zeros((S, D), dtype=np.float32)
    for j in range(HPC):
        knT = bf(kbT[:, 128 * j : 128 * (j + 1)].T @ ckvn)   # [128, S]
        v = bf(ckvn.T @ vbT[:, 128 * j : 128 * (j + 1)])     # [S, 128]
        qn = qn0 if j == 0 else qn1
        qp = qpe[64 * j : 64 * (j + 1)]
        scores = knT.T @ qn + kper.T @ qp         # [S(k), S(q)] -> st[k, q]
        st = scores
        kidx = np.arange(S)[:, None]
        qidx = np.arange(S)[None, :]
        p = np.exp(st) * (kidx <= qidx)
        p = bf(p)
        rs = p.sum(axis=0)                        # [q]
        oT = (v.T @ p)                            # [128, q]
        oT = bf(oT * (1.0 / rs))
        out += oT.T @ owT[128 * j : 128 * (j + 1)]
    return out


def sim(inputs):
    shared, per_core = _prep_inputs(inputs)
    qa_n, ckvn, kper = _sim_stage1(shared, per_core)
    out = np.zeros((S, D), dtype=np.float32)
    for c in range(N_CORES):
        out += _sim_core(shared, per_core[c], qa_n, ckvn, kper)
    return out.reshape(B, S, D)


# ----------------------------------------------------------------------------
# bass program
# ----------------------------------------------------------------------------

def _split_waits(nc, max_waits=1):
    """This walrus build accepts at most one sem wait per instruction; hoist
    excess waits onto pure-wait EventSemaphore carriers just before it."""
    n_new = 0
    for f in nc.m.functions:
        for blk in f.blocks:
            new_insts = []
            for inst in blk.instructions:
                si = getattr(inst, "sync_info", None)
                waits = list(si.on_wait) if (si is not None and si.on_wait) else []
                if len(waits) > max_waits:
                    extra, keep = waits[:-max_waits], waits[-max_waits:]
                    for w in extra:
                        n_new += 1
                        carrier = mybir.InstEventSemaphore(
                            name=f"ws-{n_new}-{inst.name}",
                            engine=inst.engine,
                            ins=[],
                            outs=[],
                            sync_info=mybir.SyncInfo(on_wait=[w], on_update=[]),
                        )
                        nc.register_instruction(carrier, overwrite=True)
                        new_insts.append(carrier)
                    si.on_wait = keep
                new_insts.append(inst)
            blk.instructions = new_insts
    return n_new


def _build_nc():
    nc = bass.Bass(num_devices=N_CORES)
    hTs = nc.dram_tensor("hTs", [D, SLC], BF, kind="ExternalInput")
    qaT = nc.dram_tensor("qaT", [D, Q_LORA], BF, kind="ExternalInput")
    kvaT = nc.dram_tensor("kvaT", [D, 640], BF, kind="ExternalInput")
    qbT = nc.dram_tensor("qbT", [Q_LORA, 512], BF, kind="ExternalInput")
    kbT = nc.dram_tensor("kbT", [KV_LORA, 256], BF, kind="ExternalInput")
    vbT = nc.dram_tensor("vbT", [KV_LORA, 256], BF, kind="ExternalInput")
    owT = nc.dram_tensor("owT", [2 * VD, D], BF, kind="ExternalInput")
    cosb = nc.dram_tensor("cosb", [128, S], F32, kind="ExternalInput")
    sinb = nc.dram_tensor("sinb", [128, S], F32, kind="ExternalInput")
    cosa = nc.dram_tensor("cosa", [64, SLC], F32, kind="ExternalInput")
    sina = nc.dram_tensor("sina", [64, SLC], F32, kind="ExternalInput")
    maskb = nc.dram_tensor("maskb", [128, 1024], BF, kind="ExternalInput")
    out = nc.dram_tensor("out", [S, D], F32, kind="ExternalOutput")

    with tile.TileContext(nc) as tc:
        with tc.tile_pool(name="persist1", bufs=1) as persist1, \
             tc.tile_pool(name="dram", bufs=1, space="DRAM") as dram:
            ones_t = persist1.tile([128, 128], BF, tag="ones")
            eps_t = persist1.tile([128, 1], F32, tag="eps")
            nc.vector.memset(eps_t, EPS)
            nc.vector.memset(ones_t, 1.0)
            qn_T = [persist1.tile([128, S], BF, tag=f"qnT{h}", name=f"qnT{h}") for h in range(HPC)]
            qpeP = persist1.tile([128, S], BF, tag="qpeP")
            ckvn_t = persist1.tile([128, CV_T, S], BF, tag="ckvn")
            kperLo = persist1.tile([128, S], BF, tag="kperLo")
            kperHi = persist1.tile([128, S], BF, tag="kperHi")
            nc.vector.memset(kperLo[64:128, :], 0.0)
            nc.vector.memset(kperHi[0:64, :], 0.0)

            ag_kv_in = dram.tile([CV_T + 1, 128, SLC], BF, tag="agkvin")
            ag_kv_out = dram.tile([N_CORES, CV_T + 1, 128, SLC], BF, tag="agkvout",
                                  addr_space="Shared")
            ag_qa_in = dram.tile([QL_T, 128, SLC], BF, tag="agqain")
            ag_qa_out = dram.tile([N_CORES, QL_T, 128, SLC], BF, tag="agqaout",
                                  addr_space="Shared")

            # ------------- stage 1 (this core's 256-token slice) -------------
            with tc.tile_pool(name="qaw", bufs=1) as qaw, \
                 tc.tile_pool(name="kvw", bufs=1) as kvw, \
                 tc.tile_pool(name="hx", bufs=1) as hx, \
                 tc.tile_pool(name="csp", bufs=1) as csp, \
                 tc.tile_pool(name="cvsb", bufs=1) as cvsb, \
                 tc.tile_pool(name="qasb", bufs=1) as qasb, \
                 tc.tile_pool(name="stg", bufs=1) as stgp, \
                 tc.tile_pool(name="sq", bufs=2) as sqp, \
                 tc.tile_pool(name="nrm", bufs=2) as nrm, \
                 tc.tile_pool(name="pet", bufs=1) as pet, \
                 tc.tile_pool(name="st_ps", bufs=3, space="PSUM") as st_ps, \
                 tc.tile_pool(name="pe_ps", bufs=2, space="PSUM") as pe_psp, \
                 tc.tile_pool(name="ssq_ps", bufs=1, space="PSUM") as ssq_ps, \
                 tc.tile_pool(name="ssq2_ps", bufs=1, space="PSUM") as ssq2_ps:

                kva_w = kvw.tile([128, D_T, 640], BF, tag="kvw")
                h_t = hx.tile([128, D_T, SLC], BF, tag="h")
                qa_w = qaw.tile([128, D_T, Q_LORA], BF, tag="qaw")
                for k in range(D_T):
                    nc.sync.dma_start(out=kva_w[:, k, :], in_=kvaT[128 * k : 128 * (k + 1), :])
                    nc.sync.dma_start(out=h_t[:, k, :], in_=hTs[128 * k : 128 * (k + 1), :])
                cos_a = csp.tile([64, SLC], F32, tag="cosa")
                sin_a = csp.tile([64, SLC], F32, tag="sina")
                nc.sync.dma_start(out=cos_a, in_=cosa[:, :])
                nc.sync.dma_start(out=sin_a, in_=sina[:, :])
                for k in range(D_T):
                    nc.sync.dma_start(out=qa_w[:, k, :], in_=qaT[128 * k : 128 * (k + 1), :])

                # ---- kv_a: 4 ckv m-tiles + kpe + kpe2 ----
                cv_t = cvsb.tile([128, CV_T, SLC], BF, tag="cv")
                stgkv = stgp.tile([128, CV_T + 1, SLC], BF, tag="stgkv")
                nc.vector.memset(stgkv[64:128, CV_T, :], 0.0)
                ssc = ssq2_ps.tile([128, SLC], F32, tag="ssc")
                pe_ps = []
                for m in range(6):
                    mp = 128 if m < 4 else 64
                    col = slice(128 * m, 128 * m + 128) if m < 4 else \
                        slice(512 + 64 * (m - 4), 512 + 64 * (m - 3))
                    if m < 4:
                        ps = st_ps.tile([mp, SLC], F32, tag="stps")
                    else:
                        ps = pe_psp.tile([mp, SLC], F32, tag="peps")
                    for k in range(D_T):
                        nc.tensor.matmul(
                            ps,
                            kva_w[:, k, col],
                            h_t[:, k, :],
                            start=(k == 0),
                            stop=(k == D_T - 1),
                        )
                    if m < 4:
                        nc.vector.tensor_copy(cv_t[:, m, :], ps)
                        sq = sqp.tile([128, SLC], BF, tag="sq")
                        nc.scalar.activation(out=sq, in_=ps, func=AF.Square)
                        nc.tensor.matmul(
                            ssc, ones_t, sq, start=(m == 0), stop=(m == CV_T - 1)
                        )
                    else:
                        pe_ps.append(ps)

                # ---- kv norm + kpe rope + stage + gather ----
                bc2 = nrm.tile([128, SLC], F32, tag="bc2")
                nc.scalar.activation(
                    out=bc2, in_=ssc, func=AF.Sqrt, scale=1.0 / KV_LORA, bias=eps_t
                )
                nc.vector.reciprocal(bc2, bc2)
                for i in range(CV_T):
                    nc.vector.tensor_mul(stgkv[:, i, :], cv_t[:, i, :], bc2)
                t1 = pet.tile([64, SLC], F32, tag="t1")
                t2 = pet.tile([64, SLC], F32, tag="t2")
                nc.vector.tensor_mul(t1, pe_ps[0], cos_a)
                nc.vector.tensor_mul(t2, pe_ps[1], sin_a)
                nc.vector.tensor_add(stgkv[0:64, CV_T, :], t1, t2)
                for s2 in range(CV_T + 1):
                    nc.sync.dma_start(out=ag_kv_in[s2], in_=stgkv[:, s2, :])
                nc.gpsimd.collective_compute(
                    "AllGather",
                    mybir.AluOpType.bypass,
                    replica_groups=[list(range(N_CORES))],
                    ins=[ag_kv_in[:].opt()],
                    outs=[ag_kv_out[:].opt()],
                )

                # ---- q_a: 12 m-tiles ----
                qa_t = qasb.tile([128, QL_T, SLC], BF, tag="qa")
                stgqa = stgp.tile([128, QL_T, SLC], BF, tag="stgqa")
                ssq = ssq_ps.tile([128, SLC], F32, tag="ssq")
                for m in range(QL_T):
                    ps = st_ps.tile([128, SLC], F32, tag="stps")
                    for k in range(D_T):
                        nc.tensor.matmul(
                            ps,
                            qa_w[:, k, 128 * m : 128 * (m + 1)],
                            h_t[:, k, :],
                            start=(k == 0),
                            stop=(k == D_T - 1),
                        )
                    nc.vector.tensor_copy(qa_t[:, m, :], ps)
                    sq = sqp.tile([128, SLC], BF, tag="sq")
                    nc.scalar.activation(out=sq, in_=ps, func=AF.Square)
                    nc.tensor.matmul(
                        ssq, ones_t, sq, start=(m == 0), stop=(m == QL_T - 1)
                    )
                bc = nrm.tile([128, SLC], F32, tag="bc")
                nc.scalar.activation(
                    out=bc, in_=ssq, func=AF.Sqrt, scale=1.0 / Q_LORA, bias=eps_t
                )
                nc.vector.reciprocal(bc, bc)
                for m in range(QL_T):
                    nc.vector.tensor_mul(stgqa[:, m, :], qa_t[:, m, :], bc)
                    nc.sync.dma_start(out=ag_qa_in[m], in_=stgqa[:, m, :])
                nc.gpsimd.collective_compute(
                    "AllGather",
                    mybir.AluOpType.bypass,
                    replica_groups=[list(range(N_CORES))],
                    ins=[ag_qa_in[:].opt()],
                    outs=[ag_qa_out[:].opt()],
                )

            # ---------------- phase B: kv_b + q_b projections ----------------
            with tc.tile_pool(name="persist2", bufs=1) as persist2:
                kn_T = [persist2.tile([128, S], BF, tag=f"knT{h}", name=f"knT{h}") for h in range(HPC)]
                v_sb = [persist2.tile([128, S], BF, tag=f"v{h}", name=f"v{h}") for h in range(HPC)]
                o_T = [persist2.tile([128, S], BF, tag=f"oT{h}", name=f"oT{h}") for h in range(HPC)]
                with tc.tile_pool(name="kbw", bufs=1) as kbw, \
                     tc.tile_pool(name="qat", bufs=1) as qat, \
                     tc.tile_pool(name="qbw", bufs=1) as qbw, \
                     tc.tile_pool(name="csp2", bufs=2) as csp2, \
                     tc.tile_pool(name="pet2", bufs=2) as pet2, \
                     tc.tile_pool(name="kn_ps", bufs=2, space="PSUM") as kn_ps, \
                     tc.tile_pool(name="v_ps", bufs=3, space="PSUM") as v_ps, \
                     tc.tile_pool(name="qt_ps", bufs=3, space="PSUM") as qt_ps:
                    # gather-back: ckv path
                    for r in range(N_CORES):
                        sl = slice(SLC * r, SLC * (r + 1))
                        nc.sync.dma_start(
                            out=ckvn_t[:, :, sl],
                            in_=ag_kv_out[r, 0:CV_T].rearrange("c p t -> p c t"),
                        )
                        nc.sync.dma_start(out=kperLo[0:64, sl], in_=ag_kv_out[r, CV_T, 0:64, :])
                        nc.sync.dma_start(out=kperHi[64:128, sl], in_=ag_kv_out[r, CV_T, 0:64, :])
                    kb_w = kbw.tile([128, CV_T, 256], BF, tag="kbw")
                    vb_w = kbw.tile([128, CV_T, 256], BF, tag="vbw")
                    for ct in range(CV_T):
                        nc.sync.dma_start(out=kb_w[:, ct, :], in_=kbT[128 * ct : 128 * (ct + 1), :])
                        nc.sync.dma_start(out=vb_w[:, ct, :], in_=vbT[128 * ct : 128 * (ct + 1), :])
                    # kv_b projections (overlap the q-path gather)
                    for h in range(HPC):
                        hs = slice(128 * h, 128 * (h + 1))
                        for c in range(NCHUNK):
                            cs = slice(NQ * c, NQ * (c + 1))
                            ps = kn_ps.tile([128, NQ], F32, tag="knps")
                            for ct in range(CV_T):
                                nc.tensor.matmul(
                                    ps,
                                    kb_w[:, ct, hs],
                                    ckvn_t[:, ct, cs],
                                    start=(ct == 0),
                                    stop=(ct == CV_T - 1),
                                )
                            nc.vector.tensor_copy(kn_T[h][:, cs], ps)
                        for kt in range(KT):
                            ks = slice(128 * kt, 128 * (kt + 1))
                            ps = v_ps.tile([128, VD], F32, tag="vps")
                            for ct in range(CV_T):
                                nc.tensor.matmul(
                                    ps,
                                    ckvn_t[:, ct, ks],
                                    vb_w[:, ct, hs],
                                    start=(ct == 0),
                                    stop=(ct == CV_T - 1),
                                )
                            nc.vector.tensor_copy(v_sb[h][:, ks], ps)

                    # gather-back: q path
                    qa_f = qat.tile([128, QL_T, S], BF, tag="qaf")
                    for r in range(N_CORES):
                        sl = slice(SLC * r, SLC * (r + 1))
                        nc.sync.dma_start(
                            out=qa_f[:, :, sl],
                            in_=ag_qa_out[r].rearrange("m p t -> p m t"),
                        )
                    qb_w = qbw.tile([128, QL_T, 512], BF, tag="qbw")
                    for m in range(QL_T):
                        nc.sync.dma_start(out=qb_w[:, m, :], in_=qbT[128 * m : 128 * (m + 1), :])
                    # q_b: 4 col-blocks accumulated over 12 m
                    for c in range(NCHUNK):
                        cs = slice(NQ * c, NQ * (c + 1))
                        cos_c = csp2.tile([128, NQ], F32, tag="cosc")
                        sin_c = csp2.tile([128, NQ], F32, tag="sinc")
                        nc.sync.dma_start(out=cos_c, in_=cosb[:, cs])
                        nc.sync.dma_start(out=sin_c, in_=sinb[:, cs])
                        qt_tiles = []
                        for b in range(4):
                            ps = qt_ps.tile([128, NQ], F32, tag="qtps")
                            for m in range(QL_T):
                                nc.tensor.matmul(
                                    ps,
                                    qb_w[:, m, 128 * b : 128 * (b + 1)],
                                    qa_f[:, m, cs],
                                    start=(m == 0),
                                    stop=(m == QL_T - 1),
                                )
                            if b == 0:
                                nc.vector.tensor_copy(qn_T[0][:, cs], ps)
                            elif b == 1:
                                nc.vector.tensor_copy(qn_T[1][:, cs], ps)
                            else:
                                qt_tiles.append(ps)
                        t1 = pet2.tile([128, NQ], F32, tag="t1")
                        t2 = pet2.tile([128, NQ], F32, tag="t2")
                        nc.vector.tensor_mul(t1, qt_tiles[0], cos_c)
                        nc.vector.tensor_mul(t2, qt_tiles[1], sin_c)
                        nc.vector.tensor_add(qpeP[:, cs], t1, t2)

                # ---------------- phase C: attention ----------------
                mskp_cm = tc.tile_pool(name="mskp", bufs=1)
                oww_cm = tc.tile_pool(name="oww", bufs=1)
                mskp = mskp_cm.__enter__()
                oww = oww_cm.__enter__()
                with tc.tile_pool(name="pp", bufs=6) as pp, \
                     tc.tile_pool(name="ep", bufs=3) as ep, \
                     tc.tile_pool(name="rvp", bufs=2) as rvp, \
                     tc.tile_pool(name="ostg", bufs=4) as ostg, \
                     tc.tile_pool(name="s_ps", bufs=3, space="PSUM") as s_ps, \
                     tc.tile_pool(name="rs_ps", bufs=2, space="PSUM") as rs_ps, \
                     tc.tile_pool(name="o_ps", bufs=2, space="PSUM") as o_ps, \
                     tc.tile_pool(name="out_ps", bufs=1, space="PSUM") as out_ps:
                    mask_s = mskp.tile([128, 1024], BF, tag="mask")
                    nc.sync.dma_start(out=mask_s, in_=maskb[:, :])
                    ow_t = oww.tile([128, HPC, D], BF, tag="oww")
                    for j in range(HPC):
                        nc.sync.dma_start(out=ow_t[:, j, :], in_=owT[128 * j : 128 * (j + 1), :])
                    for c in range(NCHUNK):
                        cs = slice(NQ * c, NQ * (c + 1))
                        nkt = 4 * (c + 1)
                        for h in range(HPC):
                            kper_h = kperLo if h == 0 else kperHi
                            rs = rs_ps.tile([128, NQ], F32, tag="rs")
                            op = o_ps.tile([128, NQ], F32, tag="op")
                            for kt in range(nkt):
                                ks = slice(128 * kt, 128 * (kt + 1))
                                i = kt - 4 * c
                                lo = 128 * i if i > 0 else 0  # valid q-subrange start
                                qs = slice(NQ * c + lo, NQ * (c + 1))
                                vs = slice(lo, NQ)
                                sp = s_ps.tile([128, NQ], F32, tag="sp")
                                nc.tensor.matmul(
                                    sp[:, vs], kn_T[h][:, ks], qn_T[h][:, qs],
                                    start=True, stop=False,
                                )
                                nc.tensor.matmul(
                                    sp[:, vs], kper_h[:, ks], qpeP[:, qs],
                                    start=False, stop=True,
                                )
                                p_t = pp.tile([128, NQ], BF, tag="p")
                                if kt >= 4 * c:
                                    e_t = ep.tile([128, NQ], BF, tag="e")
                                    nc.scalar.activation(out=e_t[:, vs], in_=sp[:, vs], func=AF.Exp)
                                    nc.vector.tensor_mul(
                                        p_t[:, vs], e_t[:, vs],
                                        mask_s[:, 384 : 896 - lo],
                                    )
                                else:
                                    nc.scalar.activation(out=p_t[:, vs], in_=sp[:, vs], func=AF.Exp)
                                nc.tensor.matmul(
                                    rs[:, vs], ones_t, p_t[:, vs],
                                    start=(kt == 0), stop=(kt == nkt - 1),
                                )
                                nc.tensor.matmul(
                                    op[:, vs],
                                    v_sb[h][:, ks],
                                    p_t[:, vs],
                                    start=(kt == 0), stop=(kt == nkt - 1),
                                )
                            rv = rvp.tile([128, NQ], F32, tag="rv")
                            nc.vector.reciprocal(rv, rs)
                            nc.vector.tensor_mul(o_T[h][:, cs], op, rv)
                        # o_proj for this chunk's 4 s-tiles (both heads now done;
                        # last chunk handled in a post-phase with deeper PSUM)
                        for si in range(4 * c, 4 * (c + 1) if c < NCHUNK - 1 else 4 * c):
                            ss = slice(128 * si, 128 * (si + 1))
                            for nch in range(NCHUNK):
                                ns = slice(NQ * nch, NQ * (nch + 1))
                                ps = out_ps.tile([128, NQ], F32, tag="outps")
                                for j in range(HPC):
                                    nc.tensor.matmul(
                                        ps,
                                        o_T[j][:, ss],
                                        ow_t[:, j, ns],
                                        start=(j == 0),
                                        stop=(j == HPC - 1),
                                    )
                                stg = ostg.tile([128, NQ], F32, tag="ostg")
                                nc.scalar.activation(out=stg, in_=ps, func=AF.Copy)
                                nc.sync.dma_start(out=out[ss, ns], in_=stg)
                # ---------------- final chunk o_proj ----------------
                with tc.tile_pool(name="ostg2", bufs=4) as ostg2, \
                     tc.tile_pool(name="out2_ps", bufs=4, space="PSUM") as out2_ps:
                    for si in range(4 * (NCHUNK - 1), 4 * NCHUNK):
                        ss = slice(128 * si, 128 * (si + 1))
                        for nch in range(NCHUNK):
                            ns = slice(NQ * nch, NQ * (nch + 1))
                            ps = out2_ps.tile([128, NQ], F32, tag="out2ps")
                            for j in range(HPC):
                                nc.tensor.matmul(
                                    ps,
                                    o_T[j][:, ss],
                                    ow_t[:, j, ns],
                                    start=(j == 0),
                                    stop=(j == HPC - 1),
                                )
                            stg = ostg2.tile([128, NQ], F32, tag="ostg2")
                            nc.scalar.activation(out=stg, in_=ps, func=AF.Copy)
                            nc.sync.dma_start(out=out[ss, ns], in_=stg)
                oww_cm.__exit__(None, None, None)
                mskp_cm.__exit__(None, None, None)

    _split_waits(nc)
    return nc


# ----------------------------------------------------------------------------
# entry point
# ----------------------------------------------------------------------------

def kernel(**inputs):
    global LAST_RESULTS
    shared, per_core = _prep_inputs(inputs)
    if "nc" not in _CACHE:
        _CACHE["nc"] = _build_nc()
    nc = _CACHE["nc"]
    in_maps = []
    for c in range(N_CORES):
        m = {
            "qaT": shared["qaT"],
            "kvaT": shared["kvaT"],
            "cosb": shared["cosb"],
            "sinb": shared["sinb"],
            "maskb": shared["maskb"],
            "hTs": per_core[c]["hTs"],
            "cosa": per_core[c]["cosa"],
            "sina": per_core[c]["sina"],
            "qbT": per_core[c]["qbT"],
            "kbT": per_core[c]["kbT"],
            "vbT": per_core[c]["vbT"],
            "owT": per_core[c]["owT"],
        }
        in_maps.append(m)
    res = run_bass_kernel_spmd(nc, in_maps, core_ids=list(range(N_CORES)))
    LAST_RESULTS = res
    out = np.zeros((S, D), dtype=np.float32)
    for r in res.results:
        out += r["out"]
    return out.reshape(B, S, D)


# revision 12
# speedup vs baseline: 1.6420x; 1.0491x over previous
"""DeepseekV3 MLA attention (B=1, S=2048, D=2048, H=16) on 8 trn2 NeuronCores.

Strategy (v3 -- collective-sharded stage 1, tensor-parallel attention):
  - stage 1 (q_a / kv_a low-rank projections + k-rope) is sharded over
    TOKENS: each core processes S/8 = 256 tokens with k-outer matmuls that
    stream behind the weight DMAs, then three device AllGathers replicate
    the activations: (1) ckv path, (2) raw q_a m0-5, (3) raw q_a m6-11 +
    the per-token inv-rms vector (the q normalization commutes through the
    linear q_b, so it is applied after q_b on the receiving side);
  - each core owns 2 heads: q_b / kv_b projections for them, causal
    flash-style attention (no max subtraction -- logits are O(1) here), and
    its slice of o_proj, producing a partial [S, D] output.  kv_b runs
    under AllGather (2) / (3); the first half of the q_b contraction runs
    under AllGather (3) with bf16 partial sums staged in SBUF;
  - host sums the 8 partials.

All matmuls run in bf16 (fp32 PSUM accumulation); rmsnorm stats, rope and
softmax run in fp32.  RoPE deinterleave + rotate-half are folded into the
weight layout on the host (extra "pre-swapped, sign-folded" weight columns)
so the device only does aligned elementwise mul/adds.
"""

from contextlib import ExitStack

import numpy as np
import ml_dtypes

import concourse.bass as bass
import concourse.mybir as mybir
import concourse.tile as tile
from concourse.bass_utils import run_bass_kernel_spmd

BF16 = ml_dtypes.bfloat16
F32 = mybir.dt.float32
BF = mybir.dt.bfloat16

B, S, D = 1, 2048, 2048
H = 16
N_CORES = 8
HPC = H // N_CORES  # heads per core = 2
SLC = S // N_CORES  # stage-1 token slice per core = 256
Q_LORA = 1536
KV_LORA = 512
NOPE = 128
ROPE = 64
VD = 128
QHD = NOPE + ROPE  # 192
THETA = 50000.0
EPS = 1e-6
SCALE = QHD ** (-0.5)

NQ = 512            # q-chunk (matmul free dim)
NCHUNK = S // NQ    # 4
KT = S // 128       # 16 k-tiles
QL_T = Q_LORA // 128  # 12
QH = QL_T // 2        # 6 m-tiles per q_a AllGather half
D_T = D // 128        # 16
CV_T = KV_LORA // 128  # 4
AF = mybir.ActivationFunctionType

LAST_RESULTS = None
_CACHE = {}


# ----------------------------------------------------------------------------
# host-side weight preparation
# ----------------------------------------------------------------------------

def _deint_perm():
    # deinterleave: out[j] = in[2j] (j<32), in[2(j-32)+1] (j>=32)
    p = np.empty(ROPE, dtype=np.int64)
    p[:32] = 2 * np.arange(32)
    p[32:] = 2 * np.arange(32) + 1
    return p


def _rope_tables(position_ids):
    pos = np.asarray(position_ids).reshape(-1).astype(np.float32)  # [S]
    inv_freq = (1.0 / (THETA ** (np.arange(0, ROPE, 2, dtype=np.float32) / ROPE)))
    freqs = np.outer(pos, inv_freq)  # [S, 32]
    cos32 = np.cos(freqs).T.astype(np.float32)  # [32, S]
    sin32 = np.sin(freqs).T.astype(np.float32)
    cos128 = np.tile(cos32, (4, 1))  # [128, S]
    sin128 = np.tile(sin32, (4, 1))
    return cos128, sin128


def _causal_mask_big():
    # M[dk, u] = 1 if u >= dk + 384 ; slice [:, 384-128*i : 896-128*i]
    # gives the diagonal-block mask indicator(dq >= dk + 128*i)
    dk = np.arange(128)[:, None]
    u = np.arange(1024)[None, :]
    return (u >= dk + 384).astype(BF16)


def _prep_inputs(inputs):
    hidden = np.asarray(inputs["hidden_states"], dtype=np.float32)[0]  # [S, D]
    position_ids = np.asarray(inputs["position_ids"])
    q_a_w = np.asarray(inputs["q_a_w"], dtype=np.float32)        # [1536, D]
    q_a_ln_w = np.asarray(inputs["q_a_ln_w"], dtype=np.float32)  # [1536]
    q_b_w = np.asarray(inputs["q_b_w"], dtype=np.float32)        # [H*192, 1536]
    kv_a_w = np.asarray(inputs["kv_a_w"], dtype=np.float32)      # [576, D]
    kv_a_ln_w = np.asarray(inputs["kv_a_ln_w"], dtype=np.float32)  # [512]
    kv_b_w = np.asarray(inputs["kv_b_w"], dtype=np.float32)      # [H*256, 512]
    o_w = np.asarray(inputs["o_w"], dtype=np.float32)            # [D, H*128]

    dp = _deint_perm()
    dps = dp[(np.arange(ROPE) ^ 32)]          # source index for the swapped term
    sgn = np.where(np.arange(ROPE) < 32, -1.0, 1.0).astype(np.float32)[:, None]

    hT = np.ascontiguousarray(hidden.T).astype(BF16)              # [D, S]
    shared = {}
    shared["qaT"] = np.ascontiguousarray(q_a_w.T).astype(BF16)          # [D, 1536]

    # kv_a columns: [ckv 512 | kpe 64 (deint) | kpe2 64 (swap+sign)]
    kva_cols = np.concatenate(
        [kv_a_w[:KV_LORA], kv_a_w[KV_LORA + dp], sgn * kv_a_w[KV_LORA + dps]], axis=0
    )  # [640, D]
    shared["kvaT"] = np.ascontiguousarray(kva_cols.T).astype(BF16)      # [D, 640]

    cos128, sin128 = _rope_tables(position_ids)
    shared["cosb"] = cos128
    shared["sinb"] = sin128
    shared["maskb"] = _causal_mask_big()

    # q_b with ln + scale folded
    qb = q_b_w * q_a_ln_w[None, :] * SCALE  # [H*192, 1536]
    qb = qb.reshape(H, QHD, Q_LORA)
    kvb = (kv_b_w * kv_a_ln_w[None, :]).reshape(H, NOPE + VD, KV_LORA)

    per_core = []
    for c in range(N_CORES):
        h0, h1 = HPC * c, HPC * c + 1
        nope0 = qb[h0, :NOPE]            # [128, 1536]
        nope1 = qb[h1, :NOPE]
        peP = np.concatenate([qb[h0, NOPE + dp], qb[h1, NOPE + dp]], axis=0)  # [128,...]
        pe2P = np.concatenate(
            [sgn * qb[h0, NOPE + dps], sgn * qb[h1, NOPE + dps]], axis=0
        )
        qb_cols = np.concatenate([nope0, nope1, peP, pe2P], axis=0)  # [512, 1536]
        kb_cols = np.concatenate([kvb[h0, :NOPE], kvb[h1, :NOPE]], axis=0)  # [256, 512]
        vb_cols = np.concatenate([kvb[h0, NOPE:], kvb[h1, NOPE:]], axis=0)  # [256, 512]
        o_slice = o_w[:, VD * h0 : VD * (h1 + 1)]  # [D, 256]
        ts = slice(SLC * c, SLC * (c + 1))
        per_core.append(
            {
                "hTs": np.ascontiguousarray(hT[:, ts]),                # [D, 256]
                "cosa": np.ascontiguousarray(cos128[0:64, ts]),        # [64, 256]
                "sina": np.ascontiguousarray(sin128[0:64, ts]),
                "qbT": np.ascontiguousarray(qb_cols.T).astype(BF16),   # [1536, 512]
                "kbT": np.ascontiguousarray(kb_cols.T).astype(BF16),   # [512, 256]
                "vbT": np.ascontiguousarray(vb_cols.T).astype(BF16),   # [512, 256]
                "owT": np.ascontiguousarray(o_slice.T).astype(BF16),   # [256, D]
            }
        )
    return shared, per_core


# ----------------------------------------------------------------------------
# numpy simulation of the device program (for host-side validation)
# ----------------------------------------------------------------------------

def _sim_stage1(shared, per_core):
    bf = lambda x: x.astype(BF16).astype(np.float32)
    qaT = shared["qaT"].astype(np.float32)        # [D, 1536]
    kvaT = shared["kvaT"].astype(np.float32)      # [D, 640]
    cos = shared["cosb"]
    sin = shared["sinb"]
    qa_r = np.zeros((Q_LORA, S), dtype=np.float32)
    binv = np.zeros((S,), dtype=np.float32)
    ckvn = np.zeros((KV_LORA, S), dtype=np.float32)
    kper = np.zeros((ROPE, S), dtype=np.float32)
    for c in range(N_CORES):
        ts = slice(SLC * c, SLC * (c + 1))
        hTs = per_core[c]["hTs"].astype(np.float32)
        qa = qaT.T @ hTs                          # [1536, 256]
        qab = bf(qa)
        ssq = bf(qab * qab).sum(axis=0)
        qa_r[:, ts] = qab
        binv[ts] = bf(1.0 / np.sqrt(ssq / Q_LORA + EPS))
        ckv = kvaT.T @ hTs                        # [640, 256]
        cb = bf(ckv[:KV_LORA])
        ssc = bf(cb * cb).sum(axis=0)
        invc = 1.0 / np.sqrt(ssc / KV_LORA + EPS)
        ckvn[:, ts] = bf(cb * invc)
        kpe, kpe2 = ckv[512:576], ckv[576:640]
        kper[:, ts] = bf(kpe * cos[0:64, ts] + kpe2 * sin[0:64, ts])
    return qa_r, binv, ckvn, kper


def _sim_core(shared, pc, qa_r, binv, ckvn, kper):
    bf = lambda x: x.astype(BF16).astype(np.float32)
    cos = shared["cosb"]
    sin = shared["sinb"]
    qbT = pc["qbT"].astype(np.float32)            # [1536, 512]
    kbT = pc["kbT"].astype(np.float32)
    vbT = pc["vbT"].astype(np.float32)
    owT = pc["owT"].astype(np.float32)

    # q_b in two contraction halves with a bf16 partial (matches device)
    qT1 = bf(qbT[: 128 * QH].T @ qa_r[: 128 * QH])
    qT = qT1 + qbT[128 * QH :].T @ qa_r[128 * QH :]
    qn0 = bf(qT[0:128] * binv)
    qn1 = bf(qT[128:256] * binv)
    pe, pe2 = qT[256:384], qT[384:512]
    qpe = bf((pe * cos + pe2 * sin) * binv)       # [128, S] packed (h0;h1)

    out = np.zeros((S, D), dtype=np.float32)
    for j in range(HPC):
        knT = bf(kbT[:, 128 * j : 128 * (j + 1)].T @ ckvn)   # [128, S]
        v = bf(ckvn.T @ vbT[:, 128 * j : 128 * (j + 1)])     # [S, 128]
        qn = qn0 if j == 0 else qn1
        qp = qpe[64 * j : 64 * (j + 1)]
        scores = knT.T @ qn + kper.T @ qp
        kidx = np.arange(S)[:, None]
        qidx = np.arange(S)[None, :]
        p = np.exp(scores) * (kidx <= qidx)
        p = bf(p)
        rs = p.sum(axis=0)
        oT = bf((v.T @ p) * (1.0 / rs))
        out += oT.T @ owT[128 * j : 128 * (j + 1)]
    return out


def sim(inputs):
    shared, per_core = _prep_inputs(inputs)
    qa_r, binv, ckvn, kper = _sim_stage1(shared, per_core)
    out = np.zeros((S, D), dtype=np.float32)
    for c in range(N_CORES):
        out += _sim_core(shared, per_core[c], qa_r, binv, ckvn, kper)
    return out.reshape(B, S, D)


# ----------------------------------------------------------------------------
# bass program
# ----------------------------------------------------------------------------

def _split_waits(nc, max_waits=1):
    """This walrus build accepts at most one sem wait per instruction; hoist
    excess waits onto pure-wait EventSemaphore carriers just before it."""
    n_new = 0
    for f in nc.m.functions:
        for blk in f.blocks:
            new_insts = []
            for inst in blk.instructions:
                si = getattr(inst, "sync_info", None)
                waits = list(si.on_wait) if (si is not None and si.on_wait) else []
                if len(waits) > max_waits:
                    extra, keep = waits[:-max_waits], waits[-max_waits:]
                    for w in extra:
                        n_new += 1
                        carrier = mybir.InstEventSemaphore(
                            name=f"ws-{n_new}-{inst.name}",
                            engine=inst.engine,
                            ins=[],
                            outs=[],
                            sync_info=mybir.SyncInfo(on_wait=[w], on_update=[]),
                        )
                        nc.register_instruction(carrier, overwrite=True)
                        new_insts.append(carrier)
                    si.on_wait = keep
                new_insts.append(inst)
            blk.instructions = new_insts
    return n_new


def _ag(nc, ins_ap, outs_ap):
    nc.gpsimd.collective_compute(
        "AllGather",
        mybir.AluOpType.bypass,
        replica_groups=[list(range(N_CORES))],
        ins=[ins_ap],
        outs=[outs_ap],
    )


def _build_nc():
    nc = bass.Bass(num_devices=N_CORES)
    hTs = nc.dram_tensor("hTs", [D, SLC], BF, kind="ExternalInput")
    qaT = nc.dram_tensor("qaT", [D, Q_LORA], BF, kind="ExternalInput")
    kvaT = nc.dram_tensor("kvaT", [D, 640], BF, kind="ExternalInput")
    qbT = nc.dram_tensor("qbT", [Q_LORA, 512], BF, kind="ExternalInput")
    kbT = nc.dram_tensor("kbT", [KV_LORA, 256], BF, kind="ExternalInput")
    vbT = nc.dram_tensor("vbT", [KV_LORA, 256], BF, kind="ExternalInput")
    owT = nc.dram_tensor("owT", [2 * VD, D], BF, kind="ExternalInput")
    cosb = nc.dram_tensor("cosb", [128, S], F32, kind="ExternalInput")
    sinb = nc.dram_tensor("sinb", [128, S], F32, kind="ExternalInput")
    cosa = nc.dram_tensor("cosa", [64, SLC], F32, kind="ExternalInput")
    sina = nc.dram_tensor("sina", [64, SLC], F32, kind="ExternalInput")
    maskb = nc.dram_tensor("maskb", [128, 1024], BF, kind="ExternalInput")
    out = nc.dram_tensor("out", [S, D], F32, kind="ExternalOutput")

    with ExitStack() as top:
        tc = top.enter_context(tile.TileContext(nc))
        if True:
            persist1 = top.enter_context(tc.tile_pool(name="persist1", bufs=1))
            wgt = top.enter_context(tc.tile_pool(name="wgt", bufs=1))
            dkv_i = top.enter_context(tc.tile_pool(name="dkv_i", bufs=1, space="DRAM"))
            dkv_o = top.enter_context(tc.tile_pool(name="dkv_o", bufs=1, space="DRAM"))
            dq1_i = top.enter_context(tc.tile_pool(name="dq1_i", bufs=1, space="DRAM"))
            dq1_o = top.enter_context(tc.tile_pool(name="dq1_o", bufs=1, space="DRAM"))
            dq2_i = top.enter_context(tc.tile_pool(name="dq2_i", bufs=1, space="DRAM"))
            dq2_o = top.enter_context(tc.tile_pool(name="dq2_o", bufs=1, space="DRAM"))
            ones_t = persist1.tile([128, 128], BF, tag="ones")
            eps_t = persist1.tile([128, 1], F32, tag="eps")
            nc.vector.memset(eps_t, EPS)
            nc.vector.memset(ones_t, 1.0)
            qn_T = [persist1.tile([128, S], BF, tag=f"qnT{h}", name=f"qnT{h}") for h in range(HPC)]
            qpeP = persist1.tile([128, S], BF, tag="qpeP")
            ckvn_t = persist1.tile([128, CV_T, S], BF, tag="ckvn")
            kperLo = persist1.tile([128, S], BF, tag="kperLo")
            kperHi = persist1.tile([128, S], BF, tag="kperHi")
            bc_full = persist1.tile([128, S], BF, tag="bcfull")
            nc.vector.memset(kperLo[64:128, :], 0.0)
            nc.vector.memset(kperHi[0:64, :], 0.0)

            ag_kv_in = dkv_i.tile([CV_T + 1, 128, SLC], BF, tag="agkvin")
            ag_kv_out = dkv_o.tile([N_CORES, CV_T + 1, 128, SLC], BF, tag="agkvout",
                                   addr_space="Shared")
            ag_q1_in = dq1_i.tile([QH, 128, SLC], BF, tag="agq1in")
            ag_q1_out = dq1_o.tile([N_CORES, QH, 128, SLC], BF, tag="agq1out",
                                   addr_space="Shared")
            ag_q2_in = dq2_i.tile([QH + 1, 128, SLC], BF, tag="agq2in")
            ag_q2_out = dq2_o.tile([N_CORES, QH + 1, 128, SLC], BF, tag="agq2out",
                                   addr_space="Shared")

            # phase-B weights, prefetched during stage 1
            qb_w = wgt.tile([128, QL_T, 512], BF, tag="qbw")
            kb_w = wgt.tile([128, CV_T, 256], BF, tag="kbw")
            vb_w = wgt.tile([128, CV_T, 256], BF, tag="vbw")
            ow_t = wgt.tile([128, HPC, D], BF, tag="oww")
            mask_s = wgt.tile([128, 1024], BF, tag="mask")

            # ------------- stage 1 (this core's 256-token slice) -------------
            with ExitStack() as ph_a:
                kvw = ph_a.enter_context(tc.tile_pool(name="kvw", bufs=1))
                hx = ph_a.enter_context(tc.tile_pool(name="hx", bufs=1))
                qaw = ph_a.enter_context(tc.tile_pool(name="qaw", bufs=1))
                csp = ph_a.enter_context(tc.tile_pool(name="csp", bufs=1))
                qasb = ph_a.enter_context(tc.tile_pool(name="qasb", bufs=1))
                stgp = ph_a.enter_context(tc.tile_pool(name="stg", bufs=1))
                sqp = ph_a.enter_context(tc.tile_pool(name="sq", bufs=2))
                nrm = ph_a.enter_context(tc.tile_pool(name="nrm", bufs=2))
                pet = ph_a.enter_context(tc.tile_pool(name="pet", bufs=1))

                kva_w = kvw.tile([128, D_T, 640], BF, tag="kvw")
                h_t = hx.tile([128, D_T, SLC], BF, tag="h")
                qa_w = qaw.tile([128, D_T, Q_LORA], BF, tag="qaw")
                for k in range(D_T):
                    nc.sync.dma_start(out=kva_w[:, k, :], in_=kvaT[128 * k : 128 * (k + 1), :])
                    nc.sync.dma_start(out=h_t[:, k, :], in_=hTs[128 * k : 128 * (k + 1), :])
                cos_a = csp.tile([64, SLC], F32, tag="cosa")
                sin_a = csp.tile([64, SLC], F32, tag="sina")
                nc.sync.dma_start(out=cos_a, in_=cosa[:, :])
                nc.sync.dma_start(out=sin_a, in_=sina[:, :])
                for k in range(D_T):
                    nc.sync.dma_start(out=qa_w[:, k, :], in_=qaT[128 * k : 128 * (k + 1), :])
                # prefetch phase-B weights (DMA engines idle after stage-1 loads)
                for m in range(QL_T):
                    nc.sync.dma_start(out=qb_w[:, m, :], in_=qbT[128 * m : 128 * (m + 1), :])
                for ct in range(CV_T):
                    nc.sync.dma_start(out=kb_w[:, ct, :], in_=kbT[128 * ct : 128 * (ct + 1), :])
                    nc.sync.dma_start(out=vb_w[:, ct, :], in_=vbT[128 * ct : 128 * (ct + 1), :])
                for j in range(HPC):
                    nc.sync.dma_start(out=ow_t[:, j, :], in_=owT[128 * j : 128 * (j + 1), :])
                nc.sync.dma_start(out=mask_s, in_=maskb[:, :])

                # ---- kv_a: k-outer over 16 hidden tiles ----
                stgkv = stgp.tile([128, CV_T + 1, SLC], BF, tag="stgkv")
                nc.vector.memset(stgkv[64:128, CV_T, :], 0.0)
                with ExitStack() as kv_s:
                    cv_psp = kv_s.enter_context(tc.tile_pool(name="cv_ps", bufs=1, space="PSUM"))
                    pe_psp = kv_s.enter_context(tc.tile_pool(name="pe_ps", bufs=1, space="PSUM"))
                    ssc_ps = kv_s.enter_context(tc.tile_pool(name="ssc_ps", bufs=1, space="PSUM"))
                    cv_ps = [cv_psp.tile([128, SLC], F32, tag=f"cv{m}", name=f"cv{m}") for m in range(CV_T)]
                    pe_ps = [pe_psp.tile([64, SLC], F32, tag=f"pe{m}", name=f"pe{m}") for m in range(2)]
                    ssc = ssc_ps.tile([128, SLC], F32, tag="ssc")
                    for k in range(D_T):
                        for m in range(CV_T):
                            nc.tensor.matmul(
                                cv_ps[m], kva_w[:, k, 128 * m : 128 * (m + 1)],
                                h_t[:, k, :], start=(k == 0), stop=(k == D_T - 1),
                            )
                        for m in range(2):
                            nc.tensor.matmul(
                                pe_ps[m], kva_w[:, k, 512 + 64 * m : 512 + 64 * (m + 1)],
                                h_t[:, k, :], start=(k == 0), stop=(k == D_T - 1),
                            )
                    cv_t = qasb.tile([128, CV_T, SLC], BF, tag="cv")
                    for m in range(CV_T):
                        nc.vector.tensor_copy(cv_t[:, m, :], cv_ps[m])
                        sq = sqp.tile([128, SLC], BF, tag="sq")
                        nc.scalar.activation(out=sq, in_=cv_ps[m], func=AF.Square)
                        nc.tensor.matmul(
                            ssc, ones_t, sq, start=(m == 0), stop=(m == CV_T - 1)
                        )
                    bc2 = nrm.tile([128, SLC], F32, tag="bc2")
                    nc.scalar.activation(
                        out=bc2, in_=ssc, func=AF.Sqrt, scale=1.0 / KV_LORA, bias=eps_t
                    )
                    nc.vector.reciprocal(bc2, bc2)
                    for m in range(CV_T):
                        nc.vector.tensor_mul(stgkv[:, m, :], cv_t[:, m, :], bc2)
                    t1 = pet.tile([64, SLC], F32, tag="t1")
                    t2 = pet.tile([64, SLC], F32, tag="t2")
                    nc.vector.tensor_mul(t1, pe_ps[0], cos_a)
                    nc.vector.tensor_mul(t2, pe_ps[1], sin_a)
                    nc.vector.tensor_add(stgkv[0:64, CV_T, :], t1, t2)
                for s2 in range(CV_T + 1):
                    nc.sync.dma_start(out=ag_kv_in[s2], in_=stgkv[:, s2, :])
                _ag(nc, ag_kv_in[:].opt(), ag_kv_out[:].opt())

                # ---- q_a: two k-outer passes of 6 m-tiles each ----
                qa_t = qasb.tile([128, QL_T, SLC], BF, tag="qa")
                with ExitStack() as qa_s:
                    qa_psp = qa_s.enter_context(tc.tile_pool(name="qa_ps", bufs=1, space="PSUM"))
                    ssq_psp = qa_s.enter_context(tc.tile_pool(name="ssq_ps", bufs=1, space="PSUM"))
                    ssq = ssq_psp.tile([128, SLC], F32, tag="ssq")
                    for half in range(2):
                        qa_ps = [qa_psp.tile([128, SLC], F32, tag=f"qa{m}", name=f"qaps{half}{m}") for m in range(QH)]
                        for k in range(D_T):
                            for m in range(QH):
                                g = QH * half + m
                                nc.tensor.matmul(
                                    qa_ps[m], qa_w[:, k, 128 * g : 128 * (g + 1)],
                                    h_t[:, k, :], start=(k == 0), stop=(k == D_T - 1),
                                )
                        for m in range(QH):
                            g = QH * half + m
                            nc.vector.tensor_copy(qa_t[:, g, :], qa_ps[m])
                        if half == 0:
                            for m in range(QH):
                                nc.sync.dma_start(out=ag_q1_in[m], in_=qa_t[:, m, :])
                            _ag(nc, ag_q1_in[:].opt(), ag_q1_out[:].opt())
                    for g in range(QL_T):
                        sq = sqp.tile([128, SLC], BF, tag="sq")
                        nc.scalar.activation(out=sq, in_=qa_t[:, g, :], func=AF.Square)
                        nc.tensor.matmul(
                            ssq, ones_t, sq, start=(g == 0), stop=(g == QL_T - 1)
                        )
                    bcq = nrm.tile([128, SLC], F32, tag="bcq")
                    nc.scalar.activation(
                        out=bcq, in_=ssq, func=AF.Sqrt, scale=1.0 / Q_LORA, bias=eps_t
                    )
                    nc.vector.reciprocal(bcq, bcq)
                    bcb = stgp.tile([128, SLC], BF, tag="bcb")
                    nc.vector.tensor_copy(bcb, bcq)
                for m in range(QH):
                    nc.sync.dma_start(out=ag_q2_in[m], in_=qa_t[:, QH + m, :])
                nc.sync.dma_start(out=ag_q2_in[QH], in_=bcb)
                _ag(nc, ag_q2_in[:].opt(), ag_q2_out[:].opt())

            # ---------------- phase B: kv_b + q_b projections ----------------
            with ExitStack() as ph_b:
                persist2 = ph_b.enter_context(tc.tile_pool(name="persist2", bufs=1))
                kn_T = [persist2.tile([128, S], BF, tag=f"knT{h}", name=f"knT{h}") for h in range(HPC)]
                v2_sb = persist2.tile([128, KT, 2 * VD], BF, tag="v2")
                o_T = [persist2.tile([128, S], BF, tag=f"oT{h}", name=f"oT{h}") for h in range(HPC)]
                with ExitStack() as qb_s:
                    qaf = qb_s.enter_context(tc.tile_pool(name="qaf", bufs=1))
                    qpartp = qb_s.enter_context(tc.tile_pool(name="qpart", bufs=1))
                    csp2 = qb_s.enter_context(tc.tile_pool(name="csp2", bufs=2))
                    pet2 = qb_s.enter_context(tc.tile_pool(name="pet2", bufs=2))
                    kn_ps = qb_s.enter_context(tc.tile_pool(name="kn_ps", bufs=2, space="PSUM"))
                    v_ps = qb_s.enter_context(tc.tile_pool(name="v_ps", bufs=3, space="PSUM"))
                    qt_ps = qb_s.enter_context(tc.tile_pool(name="qt_ps", bufs=3, space="PSUM"))
                    # gather-back: ckv path
                    for r in range(N_CORES):
                        sl = slice(SLC * r, SLC * (r + 1))
                        nc.sync.dma_start(
                            out=ckvn_t[:, :, sl],
                            in_=ag_kv_out[r, 0:CV_T].rearrange("c p t -> p c t"),
                        )
                        nc.sync.dma_start(out=kperLo[0:64, sl], in_=ag_kv_out[r, CV_T, 0:64, :])
                        nc.sync.dma_start(out=kperHi[64:128, sl], in_=ag_kv_out[r, CV_T, 0:64, :])
                    # kv_b projections (overlap the q-path gathers)
                    for h in range(HPC):
                        hs = slice(128 * h, 128 * (h + 1))
                        for c in range(NCHUNK):
                            cs = slice(NQ * c, NQ * (c + 1))
                            ps = kn_ps.tile([128, NQ], F32, tag="knps")
                            for ct in range(CV_T):
                                nc.tensor.matmul(
                                    ps,
                                    kb_w[:, ct, hs],
                                    ckvn_t[:, ct, cs],
                                    start=(ct == 0),
                                    stop=(ct == CV_T - 1),
                                )
                            nc.vector.tensor_copy(kn_T[h][:, cs], ps)
                    for kt in range(KT):
                        ks = slice(128 * kt, 128 * (kt + 1))
                        ps = v_ps.tile([128, 2 * VD], F32, tag="vps")
                        for ct in range(CV_T):
                            nc.tensor.matmul(
                                ps,
                                ckvn_t[:, ct, ks],
                                vb_w[:, ct, :],
                                start=(ct == 0),
                                stop=(ct == CV_T - 1),
                            )
                        nc.vector.tensor_copy(v2_sb[:, kt, :], ps)

                    # gather-back: q path half 1, then q_b half-1 partial sums
                    qa_f = qaf.tile([128, QL_T, S], BF, tag="qaf")
                    for r in range(N_CORES):
                        sl = slice(SLC * r, SLC * (r + 1))
                        nc.sync.dma_start(
                            out=qa_f[:, 0:QH, sl],
                            in_=ag_q1_out[r].rearrange("m p t -> p m t"),
                        )
                    qpart = qpartp.tile([128, NCHUNK, 4, NQ], BF, tag="qpart")
                    for c in range(NCHUNK):
                        cs = slice(NQ * c, NQ * (c + 1))
                        for b in range(4):
                            ps = qt_ps.tile([128, NQ], F32, tag="qtps")
                            for m in range(QH):
                                nc.tensor.matmul(
                                    ps,
                                    qb_w[:, m, 128 * b : 128 * (b + 1)],
                                    qa_f[:, m, cs],
                                    start=(m == 0),
                                    stop=(m == QH - 1),
                                )
                            nc.vector.tensor_copy(qpart[:, c, b, :], ps)

                    # gather-back: q path half 2 + inv-rms, then q_b half 2
                    for r in range(N_CORES):
                        sl = slice(SLC * r, SLC * (r + 1))
                        nc.sync.dma_start(
                            out=qa_f[:, QH:QL_T, sl],
                            in_=ag_q2_out[r, 0:QH].rearrange("m p t -> p m t"),
                        )
                        nc.sync.dma_start(out=bc_full[:, sl], in_=ag_q2_out[r, QH])
                    for c in range(NCHUNK):
                        cs = slice(NQ * c, NQ * (c + 1))
                        cos_c = csp2.tile([128, NQ], F32, tag="cosc")
                        sin_c = csp2.tile([128, NQ], F32, tag="sinc")
                        nc.sync.dma_start(out=cos_c, in_=cosb[:, cs])
                        nc.sync.dma_start(out=sin_c, in_=sinb[:, cs])
                        qt_tiles = []
                        for b in range(4):
                            ps = qt_ps.tile([128, NQ], F32, tag="qtps")
                            for m in range(QH):
                                nc.tensor.matmul(
                                    ps,
                                    qb_w[:, QH + m, 128 * b : 128 * (b + 1)],
                                    qa_f[:, QH + m, cs],
                                    start=(m == 0),
                                    stop=(m == QH - 1),
                                )
                            t = pet2.tile([128, NQ], F32, tag=f"qsum{b % 2}")
                            nc.vector.tensor_add(t, ps, qpart[:, c, b, :])
                            if b < 2:
                                nc.vector.tensor_mul(qn_T[b][:, cs], t, bc_full[:, cs])
                            else:
                                qt_tiles.append(t)
                        t1 = pet2.tile([128, NQ], F32, tag="t1")
                        t2 = pet2.tile([128, NQ], F32, tag="t2")
                        nc.vector.tensor_mul(t1, qt_tiles[0], cos_c)
                        nc.vector.tensor_mul(t2, qt_tiles[1], sin_c)
                        nc.vector.tensor_add(t1, t1, t2)
                        nc.vector.tensor_mul(qpeP[:, cs], t1, bc_full[:, cs])

                # ---------------- phase C: attention ----------------
                with ExitStack() as at_s:
                    pp = at_s.enter_context(tc.tile_pool(name="pp", bufs=6))
                    ep = at_s.enter_context(tc.tile_pool(name="ep", bufs=3))
                    rvp = at_s.enter_context(tc.tile_pool(name="rvp", bufs=2))
                    ostg = at_s.enter_context(tc.tile_pool(name="ostg", bufs=4))
                    s_ps = at_s.enter_context(tc.tile_pool(name="s_ps", bufs=3, space="PSUM"))
                    rs_ps = at_s.enter_context(tc.tile_pool(name="rs_ps", bufs=2, space="PSUM"))
                    o_ps = at_s.enter_context(tc.tile_pool(name="o_ps", bufs=2, space="PSUM"))
                    out_ps = at_s.enter_context(tc.tile_pool(name="out_ps", bufs=1, space="PSUM"))
                    for c in range(NCHUNK):
                        cs = slice(NQ * c, NQ * (c + 1))
                        nkt = 4 * (c + 1)
                        for h in range(HPC):
                            kper_h = kperLo if h == 0 else kperHi
                            rs = rs_ps.tile([128, NQ], F32, tag="rs")
                            op = o_ps.tile([128, NQ], F32, tag="op")
                            for kt in range(nkt):
                                ks = slice(128 * kt, 128 * (kt + 1))
                                i = kt - 4 * c
                                lo = 128 * i if i > 0 else 0  # valid q-subrange start
                                qs = slice(NQ * c + lo, NQ * (c + 1))
                                vs = slice(lo, NQ)
                                sp = s_ps.tile([128, NQ], F32, tag="sp")
                                nc.tensor.matmul(
                                    sp[:, vs], kn_T[h][:, ks], qn_T[h][:, qs],
                                    start=True, stop=False,
                                )
                                nc.tensor.matmul(
                                    sp[:, vs], kper_h[:, ks], qpeP[:, qs],
                                    start=False, stop=True,
                                )
                                p_t = pp.tile([128, NQ], BF, tag="p")
                                if kt >= 4 * c:
                                    e_t = ep.tile([128, NQ], BF, tag="e")
                                    nc.scalar.activation(out=e_t[:, vs], in_=sp[:, vs], func=AF.Exp)
                                    nc.vector.tensor_mul(
                                        p_t[:, vs], e_t[:, vs],
                                        mask_s[:, 384 : 896 - lo],
                                    )
                                else:
                                    nc.scalar.activation(out=p_t[:, vs], in_=sp[:, vs], func=AF.Exp)
                                nc.tensor.matmul(
                                    rs[:, vs], ones_t, p_t[:, vs],
                                    start=(kt == 0), stop=(kt == nkt - 1),
                                )
                                nc.tensor.matmul(
                                    op[:, vs],
                                    v2_sb[:, kt, 128 * h : 128 * (h + 1)],
                                    p_t[:, vs],
                                    start=(kt == 0), stop=(kt == nkt - 1),
                                )
                            rv = rvp.tile([128, NQ], F32, tag="rv")
                            nc.vector.reciprocal(rv, rs)
                            nc.vector.tensor_mul(o_T[h][:, cs], op, rv)
                        # o_proj for this chunk's 4 s-tiles (both heads now done;
                        # last chunk handled in a post-phase with deeper PSUM)
                        for si in range(4 * c, 4 * (c + 1) if c < NCHUNK - 1 else 4 * c):
                            ss = slice(128 * si, 128 * (si + 1))
                            for nch in range(NCHUNK):
                                ns = slice(NQ * nch, NQ * (nch + 1))
                                ps = out_ps.tile([128, NQ], F32, tag="outps")
                                for j in range(HPC):
                                    nc.tensor.matmul(
                                        ps,
                                        o_T[j][:, ss],
                                        ow_t[:, j, ns],
                                        start=(j == 0),
                                        stop=(j == HPC - 1),
                                    )
                                stg = ostg.tile([128, NQ], F32, tag="ostg")
                                nc.scalar.activation(out=stg, in_=ps, func=AF.Copy)
                                nc.sync.dma_start(out=out[ss, ns], in_=stg)
                # ---------------- final chunk o_proj ----------------
                with ExitStack() as fo_s:
                    ostg2 = fo_s.enter_context(tc.tile_pool(name="ostg2", bufs=4))
                    out2_ps = fo_s.enter_context(tc.tile_pool(name="out2_ps", bufs=4, space="PSUM"))
                    for si in range(4 * (NCHUNK - 1), 4 * NCHUNK):
                        ss = slice(128 * si, 128 * (si + 1))
                        for nch in range(NCHUNK):
                            ns = slice(NQ * nch, NQ * (nch + 1))
                            ps = out2_ps.tile([128, NQ], F32, tag="out2ps")
                            for j in range(HPC):
                                nc.tensor.matmul(
                                    ps,
                                    o_T[j][:, ss],
                                    ow_t[:, j, ns],
                                    start=(j == 0),
                                    stop=(j == HPC - 1),
                                )
                            stg = ostg2.tile([128, NQ], F32, tag="ostg2")
                            nc.scalar.activation(out=stg, in_=ps, func=AF.Copy)
                            nc.sync.dma_start(out=out[ss, ns], in_=stg)

    _split_waits(nc)
    return nc


# ----------------------------------------------------------------------------
# entry point
# ----------------------------------------------------------------------------

def kernel(**inputs):
    global LAST_RESULTS
    shared, per_core = _prep_inputs(inputs)
    if "nc" not in _CACHE:
        _CACHE["nc"] = _build_nc()
    nc = _CACHE["nc"]
    in_maps = []
    for c in range(N_CORES):
        m = {
            "qaT": shared["qaT"],
            "kvaT": shared["kvaT"],
            "cosb": shared["cosb"],
            "sinb": shared["sinb"],
            "maskb": shared["maskb"],
            "hTs": per_core[c]["hTs"],
            "cosa": per_core[c]["cosa"],
            "sina": per_core[c]["sina"],
            "qbT": per_core[c]["qbT"],
            "kbT": per_core[c]["kbT"],
            "vbT": per_core[c]["vbT"],
            "owT": per_core[c]["owT"],
        }
        in_maps.append(m)
    res = run_bass_kernel_spmd(nc, in_maps, core_ids=list(range(N_CORES)))
    LAST_RESULTS = res
    out = np.zeros((S, D), dtype=np.float32)
    for r in res.results:
        out += r["out"]
    return out.reshape(B, S, D)


# revision 27
# speedup vs baseline: 1.6661x; 1.0147x over previous
"""DeepseekV3 MLA attention (B=1, S=2048, D=2048, H=16) on 8 trn2 NeuronCores.

Strategy (v3 -- collective-sharded stage 1, tensor-parallel attention):
  - stage 1 (q_a / kv_a low-rank projections + k-rope) is sharded over
    TOKENS: each core processes S/8 = 256 tokens with k-outer matmuls that
    stream behind the weight DMAs, then three device AllGathers replicate
    the activations: (1) ckv path, (2) raw q_a m0-5, (3) raw q_a m6-11 +
    the per-token inv-rms vector (the q normalization commutes through the
    linear q_b, so it is applied after q_b on the receiving side);
  - each core owns 2 heads: q_b / kv_b projections for them, causal
    flash-style attention (no max subtraction -- logits are O(1) here), and
    its slice of o_proj, producing a partial [S, D] output.  kv_b runs
    under AllGather (2) / (3); the first half of the q_b contraction runs
    under AllGather (3) with bf16 partial sums staged in SBUF;
  - host sums the 8 partials.

All matmuls run in bf16 (fp32 PSUM accumulation); rmsnorm stats, rope and
softmax run in fp32.  RoPE deinterleave + rotate-half are folded into the
weight layout on the host (extra "pre-swapped, sign-folded" weight columns)
so the device only does aligned elementwise mul/adds.
"""

from contextlib import ExitStack

import numpy as np
import ml_dtypes

import concourse.bass as bass
import concourse.mybir as mybir
import concourse.tile as tile
from concourse.bass_utils import run_bass_kernel_spmd

BF16 = ml_dtypes.bfloat16
F32 = mybir.dt.float32
BF = mybir.dt.bfloat16

B, S, D = 1, 2048, 2048
H = 16
N_CORES = 8
HPC = H // N_CORES  # heads per core = 2
SLC = S // N_CORES  # stage-1 token slice per core = 256
Q_LORA = 1536
KV_LORA = 512
NOPE = 128
ROPE = 64
VD = 128
QHD = NOPE + ROPE  # 192
THETA = 50000.0
EPS = 1e-6
SCALE = QHD ** (-0.5)

NQ = 512            # q-chunk (matmul free dim)
NCHUNK = S // NQ    # 4
KT = S // 128       # 16 k-tiles
QL_T = Q_LORA // 128  # 12
QH = QL_T // 2        # 6 m-tiles per q_a AllGather half
D_T = D // 128        # 16
CV_T = KV_LORA // 128  # 4
AF = mybir.ActivationFunctionType

LAST_RESULTS = None
_CACHE = {}


# ----------------------------------------------------------------------------
# host-side weight preparation
# ----------------------------------------------------------------------------

def _deint_perm():
    # deinterleave: out[j] = in[2j] (j<32), in[2(j-32)+1] (j>=32)
    p = np.empty(ROPE, dtype=np.int64)
    p[:32] = 2 * np.arange(32)
    p[32:] = 2 * np.arange(32) + 1
    return p


def _rope_tables(position_ids):
    pos = np.asarray(position_ids).reshape(-1).astype(np.float32)  # [S]
    inv_freq = (1.0 / (THETA ** (np.arange(0, ROPE, 2, dtype=np.float32) / ROPE)))
    freqs = np.outer(pos, inv_freq)  # [S, 32]
    cos32 = np.cos(freqs).T.astype(np.float32)  # [32, S]
    sin32 = np.sin(freqs).T.astype(np.float32)
    cos128 = np.tile(cos32, (4, 1))  # [128, S]
    sin128 = np.tile(sin32, (4, 1))
    return cos128, sin128


def _causal_mask_big():
    # M[dk, u] = 1 if u >= dk + 384 ; slice [:, 384-128*i : 896-128*i]
    # gives the diagonal-block mask indicator(dq >= dk + 128*i)
    dk = np.arange(128)[:, None]
    u = np.arange(1024)[None, :]
    return (u >= dk + 384).astype(BF16)


def _prep_inputs(inputs):
    hidden = np.asarray(inputs["hidden_states"], dtype=np.float32)[0]  # [S, D]
    position_ids = np.asarray(inputs["position_ids"])
    q_a_w = np.asarray(inputs["q_a_w"], dtype=np.float32)        # [1536, D]
    q_a_ln_w = np.asarray(inputs["q_a_ln_w"], dtype=np.float32)  # [1536]
    q_b_w = np.asarray(inputs["q_b_w"], dtype=np.float32)        # [H*192, 1536]
    kv_a_w = np.asarray(inputs["kv_a_w"], dtype=np.float32)      # [576, D]
    kv_a_ln_w = np.asarray(inputs["kv_a_ln_w"], dtype=np.float32)  # [512]
    kv_b_w = np.asarray(inputs["kv_b_w"], dtype=np.float32)      # [H*256, 512]
    o_w = np.asarray(inputs["o_w"], dtype=np.float32)            # [D, H*128]

    dp = _deint_perm()
    dps = dp[(np.arange(ROPE) ^ 32)]          # source index for the swapped term
    sgn = np.where(np.arange(ROPE) < 32, -1.0, 1.0).astype(np.float32)[:, None]

    hT = np.ascontiguousarray(hidden.T).astype(BF16)              # [D, S]
    shared = {}
    shared["qaT"] = np.ascontiguousarray(q_a_w.T).astype(BF16)          # [D, 1536]

    # kv_a columns: [ckv 512 | kpe 64 (deint) | kpe2 64 (swap+sign)]
    kva_cols = np.concatenate(
        [kv_a_w[:KV_LORA], kv_a_w[KV_LORA + dp], sgn * kv_a_w[KV_LORA + dps]], axis=0
    )  # [640, D]
    shared["kvaT"] = np.ascontiguousarray(kva_cols.T).astype(BF16)      # [D, 640]

    cos128, sin128 = _rope_tables(position_ids)
    shared["cosb"] = cos128
    shared["sinb"] = sin128
    shared["maskb"] = _causal_mask_big()

    # q_b with ln + scale folded
    qb = q_b_w * q_a_ln_w[None, :] * SCALE  # [H*192, 1536]
    qb = qb.reshape(H, QHD, Q_LORA)
    kvb = (kv_b_w * kv_a_ln_w[None, :]).reshape(H, NOPE + VD, KV_LORA)

    per_core = []
    for c in range(N_CORES):
        h0, h1 = HPC * c, HPC * c + 1
        nope0 = qb[h0, :NOPE]            # [128, 1536]
        nope1 = qb[h1, :NOPE]
        peP = np.concatenate([qb[h0, NOPE + dp], qb[h1, NOPE + dp]], axis=0)  # [128,...]
        pe2P = np.concatenate(
            [sgn * qb[h0, NOPE + dps], sgn * qb[h1, NOPE + dps]], axis=0
        )
        qb_cols = np.concatenate([nope0, nope1, peP, pe2P], axis=0)  # [512, 1536]
        kb_cols = np.concatenate([kvb[h0, :NOPE], kvb[h1, :NOPE]], axis=0)  # [256, 512]
        vb_cols = np.concatenate([kvb[h0, NOPE:], kvb[h1, NOPE:]], axis=0)  # [256, 512]
        o_slice = o_w[:, VD * h0 : VD * (h1 + 1)]  # [D, 256]
        ts = slice(SLC * c, SLC * (c + 1))
        per_core.append(
            {
                "hTs": np.ascontiguousarray(hT[:, ts]),                # [D, 256]
                "cosa": np.ascontiguousarray(cos128[0:64, ts]),        # [64, 256]
                "sina": np.ascontiguousarray(sin128[0:64, ts]),
                "qbT": np.ascontiguousarray(qb_cols.T).astype(BF16),   # [1536, 512]
                "kbT": np.ascontiguousarray(kb_cols.T).astype(BF16),   # [512, 256]
                "vbT": np.ascontiguousarray(vb_cols.T).astype(BF16),   # [512, 256]
                "owT": np.ascontiguousarray(o_slice.T).astype(BF16),   # [256, D]
            }
        )
    return shared, per_core


# ----------------------------------------------------------------------------
# numpy simulation of the device program (for host-side validation)
# ----------------------------------------------------------------------------

def _sim_stage1(shared, per_core):
    bf = lambda x: x.astype(BF16).astype(np.float32)
    qaT = shared["qaT"].astype(np.float32)        # [D, 1536]
    kvaT = shared["kvaT"].astype(np.float32)      # [D, 640]
    cos = shared["cosb"]
    sin = shared["sinb"]
    qa_r = np.zeros((Q_LORA, S), dtype=np.float32)
    binv = np.zeros((S,), dtype=np.float32)
    ckvn = np.zeros((KV_LORA, S), dtype=np.float32)
    kper = np.zeros((ROPE, S), dtype=np.float32)
    for c in range(N_CORES):
        ts = slice(SLC * c, SLC * (c + 1))
        hTs = per_core[c]["hTs"].astype(np.float32)
        qa = qaT.T @ hTs                          # [1536, 256]
        qab = bf(qa)
        ssq = bf(qab * qab).sum(axis=0)
        qa_r[:, ts] = qab
        binv[ts] = bf(1.0 / np.sqrt(ssq / Q_LORA + EPS))
        ckv = kvaT.T @ hTs                        # [640, 256]
        cb = bf(ckv[:KV_LORA])
        ssc = bf(cb * cb).sum(axis=0)
        invc = 1.0 / np.sqrt(ssc / KV_LORA + EPS)
        ckvn[:, ts] = bf(cb * invc)
        kpe, kpe2 = ckv[512:576], ckv[576:640]
        kper[:, ts] = bf(kpe * cos[0:64, ts] + kpe2 * sin[0:64, ts])
    return qa_r, binv, ckvn, kper


def _sim_core(shared, pc, qa_r, binv, ckvn, kper):
    bf = lambda x: x.astype(BF16).astype(np.float32)
    cos = shared["cosb"]
    sin = shared["sinb"]
    qbT = pc["qbT"].astype(np.float32)            # [1536, 512]
    kbT = pc["kbT"].astype(np.float32)
    vbT = pc["vbT"].astype(np.float32)
    owT = pc["owT"].astype(np.float32)

    # q_b in two contraction halves with a bf16 partial (matches device)
    qT1 = bf(qbT[: 128 * QH].T @ qa_r[: 128 * QH])
    qT = qT1 + qbT[128 * QH :].T @ qa_r[128 * QH :]
    qn0 = bf(qT[0:128] * binv)
    qn1 = bf(qT[128:256] * binv)
    pe, pe2 = qT[256:384], qT[384:512]
    qpe = bf((pe * cos + pe2 * sin) * binv)       # [128, S] packed (h0;h1)

    out = np.zeros((S, D), dtype=np.float32)
    for j in range(HPC):
        knT = bf(kbT[:, 128 * j : 128 * (j + 1)].T @ ckvn)   # [128, S]
        v = bf(ckvn.T @ vbT[:, 128 * j : 128 * (j + 1)])     # [S, 128]
        qn = qn0 if j == 0 else qn1
        qp = qpe[64 * j : 64 * (j + 1)]
        scores = knT.T @ qn + kper.T @ qp
        kidx = np.arange(S)[:, None]
        qidx = np.arange(S)[None, :]
        p = np.exp(scores) * (kidx <= qidx)
        p = bf(p)
        rs = p.sum(axis=0)
        oT = bf((v.T @ p) * (1.0 / rs))
        out += oT.T @ owT[128 * j : 128 * (j + 1)]
    return out


def sim(inputs):
    shared, per_core = _prep_inputs(inputs)
    qa_r, binv, ckvn, kper = _sim_stage1(shared, per_core)
    out = np.zeros((S, D), dtype=np.float32)
    for c in range(N_CORES):
        out += _sim_core(shared, per_core[c], qa_r, binv, ckvn, kper)
    return out.reshape(B, S, D)


# ----------------------------------------------------------------------------
# bass program
# ----------------------------------------------------------------------------

def _split_waits(nc, max_waits=1):
    """This walrus build accepts at most one sem wait per instruction; hoist
    excess waits onto pure-wait EventSemaphore carriers just before it."""
    n_new = 0
    for f in nc.m.functions:
        for blk in f.blocks:
            new_insts = []
            for inst in blk.instructions:
                si = getattr(inst, "sync_info", None)
                waits = list(si.on_wait) if (si is not None and si.on_wait) else []
                if len(waits) > max_waits:
                    extra, keep = waits[:-max_waits], waits[-max_waits:]
                    for w in extra:
                        n_new += 1
                        carrier = mybir.InstEventSemaphore(
                            name=f"ws-{n_new}-{inst.name}",
                            engine=inst.engine,
                            ins=[],
                            outs=[],
                            sync_info=mybir.SyncInfo(on_wait=[w], on_update=[]),
                        )
                        nc.register_instruction(carrier, overwrite=True)
                        new_insts.append(carrier)
                    si.on_wait = keep
                new_insts.append(inst)
            blk.instructions = new_insts
    return n_new


def _ag(nc, ins_ap, outs_ap):
    nc.gpsimd.collective_compute(
        "AllGather",
        mybir.AluOpType.bypass,
        replica_groups=[list(range(N_CORES))],
        ins=[ins_ap],
        outs=[outs_ap],
    )


def _build_nc():
    nc = bass.Bass(num_devices=N_CORES)
    hTs = nc.dram_tensor("hTs", [D, SLC], BF, kind="ExternalInput")
    qaT = nc.dram_tensor("qaT", [D, Q_LORA], BF, kind="ExternalInput")
    kvaT = nc.dram_tensor("kvaT", [D, 640], BF, kind="ExternalInput")
    qbT = nc.dram_tensor("qbT", [Q_LORA, 512], BF, kind="ExternalInput")
    kbT = nc.dram_tensor("kbT", [KV_LORA, 256], BF, kind="ExternalInput")
    vbT = nc.dram_tensor("vbT", [KV_LORA, 256], BF, kind="ExternalInput")
    owT = nc.dram_tensor("owT", [2 * VD, D], BF, kind="ExternalInput")
    cosb = nc.dram_tensor("cosb", [128, S], F32, kind="ExternalInput")
    sinb = nc.dram_tensor("sinb", [128, S], F32, kind="ExternalInput")
    cosa = nc.dram_tensor("cosa", [64, SLC], F32, kind="ExternalInput")
    sina = nc.dram_tensor("sina", [64, SLC], F32, kind="ExternalInput")
    maskb = nc.dram_tensor("maskb", [128, 1024], BF, kind="ExternalInput")
    out = nc.dram_tensor("out", [S, D], BF, kind="ExternalOutput")

    with ExitStack() as top:
        tc = top.enter_context(tile.TileContext(nc))
        if True:
            persist1 = top.enter_context(tc.tile_pool(name="persist1", bufs=1))
            wgt = top.enter_context(tc.tile_pool(name="wgt", bufs=1))
            dkv_i = top.enter_context(tc.tile_pool(name="dkv_i", bufs=1, space="DRAM"))
            dkv_o = top.enter_context(tc.tile_pool(name="dkv_o", bufs=1, space="DRAM"))
            dq1_i = top.enter_context(tc.tile_pool(name="dq1_i", bufs=1, space="DRAM"))
            dq1_o = top.enter_context(tc.tile_pool(name="dq1_o", bufs=1, space="DRAM"))
            dq2_i = top.enter_context(tc.tile_pool(name="dq2_i", bufs=1, space="DRAM"))
            dq2_o = top.enter_context(tc.tile_pool(name="dq2_o", bufs=1, space="DRAM"))
            ones_t = persist1.tile([128, 128], BF, tag="ones")
            eps_t = persist1.tile([128, 1], F32, tag="eps")
            nc.vector.memset(eps_t, EPS)
            nc.vector.memset(ones_t, 1.0)
            qn_T = [persist1.tile([128, S], BF, tag=f"qnT{h}", name=f"qnT{h}") for h in range(HPC)]
            qpeP = persist1.tile([128, S], BF, tag="qpeP")
            ckvn_t = persist1.tile([128, CV_T, S], BF, tag="ckvn")
            kperLo = persist1.tile([128, S], BF, tag="kperLo")
            kperHi = persist1.tile([128, S], BF, tag="kperHi")
            bc_full = persist1.tile([128, S], BF, tag="bcfull")
            nc.vector.memset(kperLo[64:128, :], 0.0)
            nc.vector.memset(kperHi[0:64, :], 0.0)

            ag_kv_in = dkv_i.tile([CV_T + 1, 128, SLC], BF, tag="agkvin")
            ag_kv_out = dkv_o.tile([N_CORES, CV_T + 1, 128, SLC], BF, tag="agkvout",
                                   addr_space="Shared")
            ag_q1_in = dq1_i.tile([QH, 128, SLC], BF, tag="agq1in")
            ag_q1_out = dq1_o.tile([N_CORES, QH, 128, SLC], BF, tag="agq1out",
                                   addr_space="Shared")
            ag_q2_in = dq2_i.tile([QH + 1, 128, SLC], BF, tag="agq2in")
            ag_q2_out = dq2_o.tile([N_CORES, QH + 1, 128, SLC], BF, tag="agq2out",
                                   addr_space="Shared")

            # phase-B weights, prefetched during stage 1
            qb_w = wgt.tile([128, QL_T, 512], BF, tag="qbw")
            kb_w = wgt.tile([128, CV_T, 256], BF, tag="kbw")
            vb_w = wgt.tile([128, CV_T, 256], BF, tag="vbw")
            ow_t = wgt.tile([128, HPC, D], BF, tag="oww")
            mask_s = wgt.tile([128, 1024], BF, tag="mask")

            # ------------- stage 1 (this core's 256-token slice) -------------
            with ExitStack() as ph_a:
                kvw = ph_a.enter_context(tc.tile_pool(name="kvw", bufs=1))
                hx = ph_a.enter_context(tc.tile_pool(name="hx", bufs=1))
                qaw = ph_a.enter_context(tc.tile_pool(name="qaw", bufs=1))
                csp = ph_a.enter_context(tc.tile_pool(name="csp", bufs=1))
                qasb = ph_a.enter_context(tc.tile_pool(name="qasb", bufs=1))
                stgp = ph_a.enter_context(tc.tile_pool(name="stg", bufs=1))
                sqp = ph_a.enter_context(tc.tile_pool(name="sq", bufs=2))
                nrm = ph_a.enter_context(tc.tile_pool(name="nrm", bufs=2))
                pet = ph_a.enter_context(tc.tile_pool(name="pet", bufs=1))

                kva_w = kvw.tile([128, D_T, 640], BF, tag="kvw")
                h_t = hx.tile([128, D_T, SLC], BF, tag="h")
                qa_w = qaw.tile([128, D_T, Q_LORA], BF, tag="qaw")
                nc.sync.dma_start(
                    out=h_t, in_=hTs[:, :].rearrange("(k p) t -> p k t", p=128)
                )
                for i in range(2):
                    nc.sync.dma_start(
                        out=kva_w[:, 8 * i : 8 * (i + 1), :],
                        in_=kvaT[1024 * i : 1024 * (i + 1), :].rearrange(
                            "(k p) c -> p k c", p=128
                        ),
                    )
                cos_a = csp.tile([64, SLC], F32, tag="cosa")
                sin_a = csp.tile([64, SLC], F32, tag="sina")
                nc.sync.dma_start(out=cos_a, in_=cosa[:, :])
                nc.sync.dma_start(out=sin_a, in_=sina[:, :])
                for i in range(4):
                    nc.sync.dma_start(
                        out=qa_w[:, 4 * i : 4 * (i + 1), :],
                        in_=qaT[512 * i : 512 * (i + 1), :].rearrange(
                            "(k p) c -> p k c", p=128
                        ),
                    )
                # prefetch phase-B weights (DMA engines idle after stage-1 loads)
                nc.sync.dma_start(
                    out=qb_w, in_=qbT[:, :].rearrange("(m p) c -> p m c", p=128)
                )
                nc.sync.dma_start(
                    out=kb_w, in_=kbT[:, :].rearrange("(c p) n -> p c n", p=128)
                )
                nc.sync.dma_start(
                    out=vb_w, in_=vbT[:, :].rearrange("(c p) n -> p c n", p=128)
                )
                nc.sync.dma_start(
                    out=ow_t, in_=owT[:, :].rearrange("(j p) n -> p j n", p=128)
                )
                nc.sync.dma_start(out=mask_s, in_=maskb[:, :])

                # ---- kv_a: k-outer over 16 hidden tiles ----
                stgkv = stgp.tile([128, CV_T + 1, SLC], BF, tag="stgkv")
                nc.vector.memset(stgkv[64:128, CV_T, :], 0.0)
                with ExitStack() as kv_s:
                    cv_psp = kv_s.enter_context(tc.tile_pool(name="cv_ps", bufs=1, space="PSUM"))
                    pe_psp = kv_s.enter_context(tc.tile_pool(name="pe_ps", bufs=1, space="PSUM"))
                    ssc_ps = kv_s.enter_context(tc.tile_pool(name="ssc_ps", bufs=1, space="PSUM"))
                    cv_ps = [cv_psp.tile([128, SLC], F32, tag=f"cv{m}", name=f"cv{m}") for m in range(CV_T)]
                    pe_ps = [pe_psp.tile([64, SLC], F32, tag=f"pe{m}", name=f"pe{m}") for m in range(2)]
                    ssc = ssc_ps.tile([128, SLC], F32, tag="ssc")
                    for k in range(D_T):
                        for m in range(CV_T):
                            nc.tensor.matmul(
                                cv_ps[m], kva_w[:, k, 128 * m : 128 * (m + 1)],
                                h_t[:, k, :], start=(k == 0), stop=(k == D_T - 1),
                            )
                        for m in range(2):
                            nc.tensor.matmul(
                                pe_ps[m], kva_w[:, k, 512 + 64 * m : 512 + 64 * (m + 1)],
                                h_t[:, k, :], start=(k == 0), stop=(k == D_T - 1),
                            )
                    cv_t = qasb.tile([128, CV_T, SLC], BF, tag="cv")
                    for m in range(CV_T):
                        nc.vector.tensor_copy(cv_t[:, m, :], cv_ps[m])
                        sq = sqp.tile([128, SLC], BF, tag="sq")
                        nc.scalar.activation(out=sq, in_=cv_ps[m], func=AF.Square)
                        nc.tensor.matmul(
                            ssc, ones_t, sq, start=(m == 0), stop=(m == CV_T - 1)
                        )
                    bc2 = nrm.tile([128, SLC], F32, tag="bc2")
                    nc.scalar.activation(
                        out=bc2, in_=ssc, func=AF.Sqrt, scale=1.0 / KV_LORA, bias=eps_t
                    )
                    nc.vector.reciprocal(bc2, bc2)
                    for m in range(CV_T):
                        nc.vector.tensor_mul(stgkv[:, m, :], cv_t[:, m, :], bc2)
                    t1 = pet.tile([64, SLC], F32, tag="t1")
                    t2 = pet.tile([64, SLC], F32, tag="t2")
                    nc.vector.tensor_mul(t1, pe_ps[0], cos_a)
                    nc.vector.tensor_mul(t2, pe_ps[1], sin_a)
                    nc.vector.tensor_add(stgkv[0:64, CV_T, :], t1, t2)
                nc.gpsimd.dma_start(
                    out=ag_kv_in[:].rearrange("s p t -> p s t"), in_=stgkv[:, :, :]
                )
                _ag(nc, ag_kv_in[:].opt(), ag_kv_out[:].opt())

                # ---- q_a: two k-outer passes of 6 m-tiles each ----
                qa_t = qasb.tile([128, QL_T, SLC], BF, tag="qa")
                with ExitStack() as qa_s:
                    qa_psp = qa_s.enter_context(tc.tile_pool(name="qa_ps", bufs=1, space="PSUM"))
                    ssq_psp = qa_s.enter_context(tc.tile_pool(name="ssq_ps", bufs=1, space="PSUM"))
                    ssq = ssq_psp.tile([128, SLC], F32, tag="ssq")
                    for half in range(2):
                        qa_ps = [qa_psp.tile([128, SLC], F32, tag=f"qa{m}", name=f"qaps{half}{m}") for m in range(QH)]
                        for k in range(D_T):
                            for m in range(QH):
                                g = QH * half + m
                                nc.tensor.matmul(
                                    qa_ps[m], qa_w[:, k, 128 * g : 128 * (g + 1)],
                                    h_t[:, k, :], start=(k == 0), stop=(k == D_T - 1),
                                )
                        for m in range(QH):
                            g = QH * half + m
                            nc.vector.tensor_copy(qa_t[:, g, :], qa_ps[m])
                        if half == 0:
                            nc.gpsimd.dma_start(
                                out=ag_q1_in[:].rearrange("m p t -> p m t"),
                                in_=qa_t[:, 0:QH, :],
                            )
                            _ag(nc, ag_q1_in[:].opt(), ag_q1_out[:].opt())
                        for m in range(QH):
                            g = QH * half + m
                            sq = sqp.tile([128, SLC], BF, tag="sq")
                            nc.scalar.activation(out=sq, in_=qa_t[:, g, :], func=AF.Square)
                            nc.tensor.matmul(
                                ssq, ones_t, sq, start=(g == 0), stop=(g == QL_T - 1)
                            )
                    bcq = nrm.tile([128, SLC], F32, tag="bcq")
                    nc.scalar.activation(
                        out=bcq, in_=ssq, func=AF.Sqrt, scale=1.0 / Q_LORA, bias=eps_t
                    )
                    nc.vector.reciprocal(bcq, bcq)
                    bcb = stgp.tile([128, SLC], BF, tag="bcb")
                    nc.vector.tensor_copy(bcb, bcq)
                nc.gpsimd.dma_start(
                    out=ag_q2_in[0:QH].rearrange("m p t -> p m t"),
                    in_=qa_t[:, QH:QL_T, :],
                )
                nc.gpsimd.dma_start(out=ag_q2_in[QH], in_=bcb)
                _ag(nc, ag_q2_in[:].opt(), ag_q2_out[:].opt())

            # ---------------- phase B: kv_b + q_b projections ----------------
            with ExitStack() as ph_b:
                persist2 = ph_b.enter_context(tc.tile_pool(name="persist2", bufs=1))
                kn_T = [persist2.tile([128, S], BF, tag=f"knT{h}", name=f"knT{h}") for h in range(HPC)]
                v2_sb = persist2.tile([128, KT, 2 * VD], BF, tag="v2")
                o_T = [persist2.tile([128, S], BF, tag=f"oT{h}", name=f"oT{h}") for h in range(HPC)]
                with ExitStack() as qb_s:
                    qaf = qb_s.enter_context(tc.tile_pool(name="qaf", bufs=1))
                    qpartp = qb_s.enter_context(tc.tile_pool(name="qpart", bufs=1))
                    csp2 = qb_s.enter_context(tc.tile_pool(name="csp2", bufs=2))
                    pet2 = qb_s.enter_context(tc.tile_pool(name="pet2", bufs=2))
                    kn_ps = qb_s.enter_context(tc.tile_pool(name="kn_ps", bufs=2, space="PSUM"))
                    v_ps = qb_s.enter_context(tc.tile_pool(name="v_ps", bufs=3, space="PSUM"))
                    qt_ps = qb_s.enter_context(tc.tile_pool(name="qt_ps", bufs=3, space="PSUM"))
                    # gather-back: ckv path (gpsimd DMA queues -- the sync
                    # queues are still draining the stage-1 weight loads)
                    for r in range(N_CORES):
                        sl = slice(SLC * r, SLC * (r + 1))
                        nc.gpsimd.dma_start(
                            out=ckvn_t[:, :, sl],
                            in_=ag_kv_out[r, 0:CV_T].rearrange("c p t -> p c t"),
                        )
                    nc.gpsimd.dma_start(
                        out=kperLo[0:64, :].rearrange("p (r t) -> p r t", r=N_CORES),
                        in_=ag_kv_out[:, CV_T, 0:64, :].rearrange("r p t -> p r t"),
                    )
                    nc.gpsimd.dma_start(
                        out=kperHi[64:128, :].rearrange("p (r t) -> p r t", r=N_CORES),
                        in_=ag_kv_out[:, CV_T, 0:64, :].rearrange("r p t -> p r t"),
                    )
                    # kv_b projections (overlap the q-path gathers)
                    for h in range(HPC):
                        hs = slice(128 * h, 128 * (h + 1))
                        for c in range(NCHUNK):
                            cs = slice(NQ * c, NQ * (c + 1))
                            ps = kn_ps.tile([128, NQ], F32, tag="knps")
                            for ct in range(CV_T):
                                nc.tensor.matmul(
                                    ps,
                                    kb_w[:, ct, hs],
                                    ckvn_t[:, ct, cs],
                                    start=(ct == 0),
                                    stop=(ct == CV_T - 1),
                                )
                            nc.vector.tensor_copy(kn_T[h][:, cs], ps)
                    for kt in range(KT):
                        ks = slice(128 * kt, 128 * (kt + 1))
                        ps = v_ps.tile([128, 2 * VD], F32, tag="vps")
                        for ct in range(CV_T):
                            nc.tensor.matmul(
                                ps,
                                ckvn_t[:, ct, ks],
                                vb_w[:, ct, :],
                                start=(ct == 0),
                                stop=(ct == CV_T - 1),
                            )
                        nc.vector.tensor_copy(v2_sb[:, kt, :], ps)

                    # gather-back: q path half 1, then q_b half-1 partial sums
                    qa_f = qaf.tile([128, QL_T, S], BF, tag="qaf")
                    for r in range(N_CORES):
                        sl = slice(SLC * r, SLC * (r + 1))
                        nc.scalar.dma_start(
                            out=qa_f[:, 0:QH, sl],
                            in_=ag_q1_out[r].rearrange("m p t -> p m t"),
                        )
                    qpart = qpartp.tile([128, NCHUNK, 4, NQ], BF, tag="qpart")
                    for c in range(NCHUNK):
                        cs = slice(NQ * c, NQ * (c + 1))
                        for b in range(4):
                            ps = qt_ps.tile([128, NQ], F32, tag="qtps")
                            for m in range(QH):
                                nc.tensor.matmul(
                                    ps,
                                    qb_w[:, m, 128 * b : 128 * (b + 1)],
                                    qa_f[:, m, cs],
                                    start=(m == 0),
                                    stop=(m == QH - 1),
                                )
                            nc.vector.tensor_copy(qpart[:, c, b, :], ps)

                    # gather-back: q path half 2 + inv-rms, then q_b half 2
                    for r in range(N_CORES):
                        sl = slice(SLC * r, SLC * (r + 1))
                        nc.scalar.dma_start(
                            out=qa_f[:, QH:QL_T, sl],
                            in_=ag_q2_out[r, 0:QH].rearrange("m p t -> p m t"),
                        )
                    nc.scalar.dma_start(
                        out=bc_full[:, :].rearrange("p (r t) -> p r t", r=N_CORES),
                        in_=ag_q2_out[:, QH].rearrange("r p t -> p r t"),
                    )
                    for c in range(NCHUNK):
                        cs = slice(NQ * c, NQ * (c + 1))
                        cos_c = csp2.tile([128, NQ], F32, tag="cosc")
                        sin_c = csp2.tile([128, NQ], F32, tag="sinc")
                        nc.sync.dma_start(out=cos_c, in_=cosb[:, cs])
                        nc.sync.dma_start(out=sin_c, in_=sinb[:, cs])
                        qt_tiles = []
                        for b in range(4):
                            ps = qt_ps.tile([128, NQ], F32, tag="qtps")
                            for m in range(QH):
                                nc.tensor.matmul(
                                    ps,
                                    qb_w[:, QH + m, 128 * b : 128 * (b + 1)],
                                    qa_f[:, QH + m, cs],
                                    start=(m == 0),
                                    stop=(m == QH - 1),
                                )
                            t = pet2.tile([128, NQ], F32, tag=f"qsum{b % 2}")
                            nc.vector.tensor_add(t, ps, qpart[:, c, b, :])
                            if b < 2:
                                nc.vector.tensor_mul(qn_T[b][:, cs], t, bc_full[:, cs])
                            else:
                                qt_tiles.append(t)
                        t1 = pet2.tile([128, NQ], F32, tag="t1")
                        t2 = pet2.tile([128, NQ], F32, tag="t2")
                        nc.vector.tensor_mul(t1, qt_tiles[0], cos_c)
                        nc.vector.tensor_mul(t2, qt_tiles[1], sin_c)
                        nc.vector.tensor_add(t1, t1, t2)
                        nc.vector.tensor_mul(qpeP[:, cs], t1, bc_full[:, cs])

                # ---------------- phase C: attention ----------------
                with ExitStack() as at_s:
                    pp = at_s.enter_context(tc.tile_pool(name="pp", bufs=6))
                    ep = at_s.enter_context(tc.tile_pool(name="ep", bufs=3))
                    rvp = at_s.enter_context(tc.tile_pool(name="rvp", bufs=2))
                    ostg = at_s.enter_context(tc.tile_pool(name="ostg", bufs=4))
                    s_ps = at_s.enter_context(tc.tile_pool(name="s_ps", bufs=3, space="PSUM"))
                    rs_ps = at_s.enter_context(tc.tile_pool(name="rs_ps", bufs=2, space="PSUM"))
                    o_ps = at_s.enter_context(tc.tile_pool(name="o_ps", bufs=2, space="PSUM"))
                    out_ps = at_s.enter_context(tc.tile_pool(name="out_ps", bufs=1, space="PSUM"))
                    for c in range(NCHUNK):
                        cs = slice(NQ * c, NQ * (c + 1))
                        nkt = 4 * (c + 1)
                        for h in range(HPC):
                            kper_h = kperLo if h == 0 else kperHi
                            rs = rs_ps.tile([128, NQ], F32, tag="rs")
                            op = o_ps.tile([128, NQ], F32, tag="op")
                            for kt in range(nkt):
                                ks = slice(128 * kt, 128 * (kt + 1))
                                i = kt - 4 * c
                                lo = 128 * i if i > 0 else 0  # valid q-subrange start
                                qs = slice(NQ * c + lo, NQ * (c + 1))
                                vs = slice(lo, NQ)
                                sp = s_ps.tile([128, NQ], F32, tag="sp")
                                nc.tensor.matmul(
                                    sp[:, vs], kn_T[h][:, ks], qn_T[h][:, qs],
                                    start=True, stop=False,
                                )
                                nc.tensor.matmul(
                                    sp[:, vs], kper_h[:, ks], qpeP[:, qs],
                                    start=False, stop=True,
                                )
                                p_t = pp.tile([128, NQ], BF, tag="p")
                                if kt >= 4 * c:
                                    e_t = ep.tile([128, NQ], BF, tag="e")
                                    nc.scalar.activation(out=e_t[:, vs], in_=sp[:, vs], func=AF.Exp)
                                    nc.vector.tensor_mul(
                                        p_t[:, vs], e_t[:, vs],
                                        mask_s[:, 384 : 896 - lo],
                                    )
                                else:
                                    nc.scalar.activation(out=p_t[:, vs], in_=sp[:, vs], func=AF.Exp)
                                nc.tensor.matmul(
                                    rs[:, vs], ones_t, p_t[:, vs],
                                    start=(kt == 0), stop=(kt == nkt - 1),
                                )
                                nc.tensor.matmul(
                                    op[:, vs],
                                    v2_sb[:, kt, 128 * h : 128 * (h + 1)],
                                    p_t[:, vs],
                                    start=(kt == 0), stop=(kt == nkt - 1),
                                )
                            rv = rvp.tile([128, NQ], F32, tag="rv")
                            nc.vector.reciprocal(rv, rs)
                            nc.vector.tensor_mul(o_T[h][:, cs], op, rv)
                        # o_proj for this chunk's 4 s-tiles (both heads now done;
                        # last chunk handled in a post-phase with deeper PSUM)
                        for si in range(4 * c, 4 * (c + 1) if c < NCHUNK - 1 else 4 * c):
                            ss = slice(128 * si, 128 * (si + 1))
                            stg = ostg.tile([128, NCHUNK, NQ], BF, tag="ostg")
                            for nch in range(NCHUNK):
                                ns = slice(NQ * nch, NQ * (nch + 1))
                                ps = out_ps.tile([128, NQ], F32, tag="outps")
                                for j in range(HPC):
                                    nc.tensor.matmul(
                                        ps,
                                        o_T[j][:, ss],
                                        ow_t[:, j, ns],
                                        start=(j == 0),
                                        stop=(j == HPC - 1),
                                    )
                                nc.scalar.activation(out=stg[:, nch, :], in_=ps, func=AF.Copy)
                            nc.sync.dma_start(
                                out=out[ss, :], in_=stg[:, :, :].rearrange("p n q -> p (n q)")
                            )
                # ---------------- final chunk o_proj ----------------
                with ExitStack() as fo_s:
                    ostg2 = fo_s.enter_context(tc.tile_pool(name="ostg2", bufs=4))
                    out2_ps = fo_s.enter_context(tc.tile_pool(name="out2_ps", bufs=4, space="PSUM"))
                    for si in range(4 * (NCHUNK - 1), 4 * NCHUNK):
                        ss = slice(128 * si, 128 * (si + 1))
                        stg = ostg2.tile([128, NCHUNK, NQ], BF, tag="ostg2")
                        for nch in range(NCHUNK):
                            ns = slice(NQ * nch, NQ * (nch + 1))
                            ps = out2_ps.tile([128, NQ], F32, tag="out2ps")
                            for j in range(HPC):
                                nc.tensor.matmul(
                                    ps,
                                    o_T[j][:, ss],
                                    ow_t[:, j, ns],
                                    start=(j == 0),
                                    stop=(j == HPC - 1),
                                )
                            nc.scalar.activation(out=stg[:, nch, :], in_=ps, func=AF.Copy)
                        nc.sync.dma_start(
                            out=out[ss, :], in_=stg[:, :, :].rearrange("p n q -> p (n q)")
                        )

    _split_waits(nc)
    return nc


# ----------------------------------------------------------------------------
# entry point
# ----------------------------------------------------------------------------

def kernel(**inputs):
    global LAST_RESULTS
    shared, per_core = _prep_inputs(inputs)
    if "nc" not in _CACHE:
        _CACHE["nc"] = _build_nc()
    nc = _CACHE["nc"]
    in_maps = []
    for c in range(N_CORES):
        m = {
            "qaT": shared["qaT"],
            "kvaT": shared["kvaT"],
            "cosb": shared["cosb"],
            "sinb": shared["sinb"],
            "maskb": shared["maskb"],
            "hTs": per_core[c]["hTs"],
            "cosa": per_core[c]["cosa"],
            "sina": per_core[c]["sina"],
            "qbT": per_core[c]["qbT"],
            "kbT": per_core[c]["kbT"],
            "vbT": per_core[c]["vbT"],
            "owT": per_core[c]["owT"],
        }
        in_maps.append(m)
    res = run_bass_kernel_spmd(nc, in_maps, core_ids=list(range(N_CORES)))
    LAST_RESULTS = res
    out = np.zeros((S, D), dtype=np.float32)
    for r in res.results:
        out += np.asarray(r["out"], dtype=np.float32)
    return out.reshape(B, S, D)
